# revision 25
# baseline (speedup 1.0000x reference)
"""Trainium2 Bass kernel for nn_GCNModel (6-layer GCN + 3-layer FC mesh deformer).

Strategy
--------
Data-parallel over batch B=32 across 8 NeuronCores (4 batch elements each).

Algebraic restructuring (host side, exact):
  ReLU only follows GCN layers 2, 4, 6, so each pair of GCN layers collapses:
      A(A x W1 + 1 b1^T) W2 + 1 b2^T
        = A^2 x (W1 W2) + (A 1) (b1 W2)^T + 1 b2^T
  with A the dense-ified normalized adjacency.  Three aggregations with a
  host-precomputed dense A^2 replace six sparse gather/scatter aggregations.
  Further:
    * pair 1's aggregation input is rank-3 (x = [verts | 1 img^T]):
      A^2 x W12 = (A^2 verts) W12[:3] + (A^2 1) (img W12[3:])^T
      so the wide aggregation reduces to a width-3 one plus rank-1 terms,
      all folded into ONE k=4 (k=6 with biases) matmul per output tile.
    * pair 3 aggregates after the [512,3] transform (width 3).
  Only pair 2 needs a full width-512 dense A^2 apply per batch element.

Everything on the critical path runs in fp8 (e4m3) DoubleRow matmuls with
fp32 PSUM accumulation where the layout permits: the A^2 aggregations, the
W34 transform, the W56 transform, and the whole FC head (weights, x3, h1,
h2 all fp8).  Host-validated vs the fp32 reference: ~6e-3 max relative
error (the output is dominated by `vertices` plus a 0.1-scaled
tanh-squashed deformation).

Layouts alternate vertex-major / feature-major so no transposes are needed
in the hot path:
  agg (contracts over vertices):  lhsT = t (vertex-major), rhs = A2T rows
                                  -> feature-major output
  transform (contracts over features): lhsT = x (feature-major), rhs = W
                                  -> vertex-major output
"""

import numpy as np
import ml_dtypes

B, V, E, IMG_F = 32, 2048, 12288, 512
N_CORES = 8
BL = B // N_CORES  # 4 batch elements per core
P = 128
NV = V // P   # 16 vertex chunks
F = 512
NF = F // P   # 4 feature chunks
FC_H = 1024
FLAT = V * 3  # 6144
NKFC1 = FLAT // P  # 48
NKFC2 = FC_H // P  # 8
NV2 = NV // 2  # 8 double-row vertex chunks
NK1DR = NKFC1 // 2  # 24 DoubleRow k-tiles for FC1
NK2DR = NKFC2 // 2  # 4 DoubleRow k-tiles for FC2/FC3

BF16 = ml_dtypes.bfloat16
FP8 = ml_dtypes.float8_e4m3

_CACHE = {}


def _host_prep(inputs):
    """Exact (fp64) host-side algebra: dense A^2, collapsed weights, shards."""
    ei = np.asarray(inputs["edge_index"])
    src = np.concatenate([ei[0], np.arange(V)]).astype(np.int64)
    dst = np.concatenate([ei[1], np.arange(V)]).astype(np.int64)
    deg = np.zeros(V)
    np.add.at(deg, dst, 1.0)
    dinv = 1.0 / np.sqrt(deg)
    normv = dinv[src] * dinv[dst]
    A = np.zeros((V, V))
    np.add.at(A, (dst, src), normv)
    A2 = A @ A
    rho = (A @ np.ones(V)).astype(np.float32)
    rho2 = (A2 @ np.ones(V)).astype(np.float32)

    W = [np.asarray(inputs[f"W{i}"], np.float64) for i in range(1, 7)]
    bb = [np.asarray(inputs[f"b{i}"], np.float64) for i in range(1, 7)]
    W12 = W[0] @ W[1]
    W34 = W[2] @ W[3]
    W56 = W[4] @ W[5]
    bias1 = bb[0] @ W[1]  # pairs with rho
    bias2 = bb[2] @ W[3]
    bias3 = bb[4] @ W[5]
    b2, b4, b6 = bb[1], bb[3], bb[5]

    shared = {}
    # A2T in fp8 DoubleRow layout: [uc2][p, j*V + v] = A2T[uc2*256+j*128+p, v]
    A2T = np.ascontiguousarray(A2.T).astype(np.float32)
    shared["A2T"] = np.ascontiguousarray(
        A2T.reshape(NV2, 2, P, V).transpose(0, 2, 1, 3).reshape(NV2, P, 2 * V)
    ).astype(FP8)
    # k=3 static lhsT rows for pair1 (verts); c1 (img term) is folded in as
    # lhsT row 3 (rhs row 3 = rho2), biases as rows 4-5 (rhs rows = rho1).
    shared["W12A"] = np.asarray(W12[:3], np.float32).astype(BF16)
    bias_pack1 = np.stack([bias1, b2]).astype(np.float32)  # pairs with rho1
    shared["HAS_BIAS1"] = bool(np.any(bias_pack1))
    shared["BIASP1"] = bias_pack1.astype(BF16)
    shared["RHO2"] = rho2.reshape(1, V).astype(BF16)
    shared["RHO1"] = np.stack([rho, np.ones(V, np.float32)]).astype(BF16)

    def pack_rows(w, ncol):
        # [nk*128, ncol] -> [128, nk*ncol] with chunk kc at cols [kc*ncol:...]
        w = np.asarray(w, np.float32)
        nk = w.shape[0] // P
        return np.ascontiguousarray(
            w.reshape(nk, P, ncol).transpose(1, 0, 2).reshape(P, nk * ncol)
        )

    def pack_dr(w, ncol):
        # [nk2*256, ncol] -> [nk2, 128, 2*ncol]: tile i, row p, col j*ncol+n
        # = w[i*256 + j*128 + p, n]  (DoubleRow k-pair layout)
        w = np.asarray(w, np.float32)
        nk2 = w.shape[0] // 256
        return np.ascontiguousarray(
            w.reshape(nk2, 2, P, ncol).transpose(0, 2, 1, 3)
            .reshape(nk2, P, 2 * ncol)
        )

    shared["W12B"] = pack_rows(W12[3:], F).astype(BF16)
    # W34 in fp8 DoubleRow layout: [p, (fc2, j, fout)] = W34[fc2*256+j*128+p, f]
    W34f = np.asarray(W34, np.float32)
    shared["W34"] = np.ascontiguousarray(
        W34f.reshape(2, 2, P, F).transpose(2, 0, 1, 3).reshape(P, 4 * F)
    ).astype(FP8)
    # W56 fp8 DoubleRow: [p, (kc2, j, c-slot16)] = W56[kc2*256+j*128+p, c]
    # (c slot padded 3->16: dual-fp8 LDW requires 16B-aligned j-stride)
    w56r = np.zeros((2, 2, P, 16), np.float32)
    w56r[:, :, :, :3] = np.asarray(W56, np.float32).reshape(2, 2, P, 3)
    shared["W56"] = np.ascontiguousarray(
        w56r.transpose(2, 0, 1, 3).reshape(P, 64)
    ).astype(FP8)

    # pair2/3 bias packs (zero in the shipped model; matmul-folded if not)
    bias_pack2 = np.stack([bias2, b4]).astype(np.float32)  # [2, 512]
    bias_pack3 = np.zeros((2, BL * 3), np.float32)
    for b in range(BL):
        bias_pack3[0, b * 3:(b + 1) * 3] = bias3
        bias_pack3[1, b * 3:(b + 1) * 3] = b6
    shared["HAS_BIAS2"] = bool(np.any(bias_pack2))
    shared["HAS_BIAS3"] = bool(np.any(bias_pack3))
    shared["BIASP2"] = bias_pack2.astype(BF16)
    shared["BIASP3"] = bias_pack3.astype(BF16)

    # FC weights, fp8 DoubleRow tiles. fcW1 rows permuted: new row
    # (vc*3+c)*128+p corresponds to original row (vc*128+p)*3+c.
    fcW1 = np.asarray(inputs["fcW1"], np.float32)
    idx = (
        (np.arange(NV)[:, None, None] * P + np.arange(P)[None, None, :]) * 3
        + np.arange(3)[None, :, None]
    ).reshape(-1)  # (vc, c, p) -> orig row
    shared["FCW1"] = pack_dr(fcW1[idx], FC_H).astype(FP8)  # [24, 128, 2048]
    shared["FCW2"] = pack_dr(
        np.asarray(inputs["fcW2"], np.float32), FC_H
    ).astype(FP8)  # [4, 128, 2048]
    # FCW3: [(ch*4+kc2), p, j*1024+n] = fcW3[kc2*256+j*128+p, ch*1024+n]
    fcW3 = np.asarray(inputs["fcW3"], np.float32)
    shared["FCW3"] = np.ascontiguousarray(
        fcW3.reshape(NK2DR, 2, P, FLAT // FC_H, FC_H)
        .transpose(3, 0, 2, 1, 4).reshape(24, P, 2 * FC_H)
    ).astype(FP8)
    fcb1 = np.asarray(inputs["fcb1"], np.float32)
    fcb2 = np.asarray(inputs["fcb2"], np.float32)
    fcb3 = np.asarray(inputs["fcb3"], np.float32)
    shared["HAS_FCB"] = bool(np.any(fcb1) or np.any(fcb2) or np.any(fcb3))
    shared["FCB1"] = np.ascontiguousarray(np.broadcast_to(fcb1, (BL, FC_H)))
    shared["FCB2"] = np.ascontiguousarray(np.broadcast_to(fcb2, (BL, FC_H)))
    shared["FCB3"] = np.ascontiguousarray(np.broadcast_to(fcb3, (BL, FLAT)))

    # W12A16DR (no-bias x1 lhsT, fp8 DoubleRow over the 16-row phase0
    # output): [b][p, j*F+f] = w12a16[b][j*8+p, f], with W12A rows at
    # g=b*4+c, zeros elsewhere; the c1 slot g=b*4+3 is filled on device.
    # The j*8+p pairing lets the avt repack DMA read plain partition
    # ranges (avt rows 0-7 -> j=0, rows 8-15 -> j=1).
    w12a16 = np.zeros((BL, 16, F), np.float32)
    for b in range(BL):
        w12a16[b, b * 4:b * 4 + 3, :] = np.asarray(W12[:3], np.float32)
    shared["W12A16DR"] = np.ascontiguousarray(
        w12a16.reshape(BL, 2, 8, F).transpose(0, 2, 1, 3)
        .reshape(BL, 8, 2 * F)
    ).astype(FP8)

    # per-core shards
    verts = np.asarray(inputs["vertices"], np.float32)  # [B, V, 3]
    img = np.asarray(inputs["img_features"], np.float32)  # [B, 512]
    per_core = []
    for c in range(N_CORES):
        vb = verts[c * BL:(c + 1) * BL]  # [BL, V, 3]
        # DoubleRow lhsT: [uc2][p, j*16 + (b*4+cc)] = verts[b, uc2*256+j*128+p, cc]
        # with a ones column at g=b*4+3 so phase0 emits rho2 = A^2 @ 1 as
        # row 3 of every batch group.  (16B-aligned j-stride for dual-fp8.)
        vraw = vb.transpose(1, 0, 2).reshape(NV2, 2, P, BL, 3)
        vvm = np.zeros((NV2, P, 2, BL, 4), np.float32)
        vvm[:, :, :, :, :3] = vraw.transpose(0, 2, 1, 3, 4)
        vvm[:, :, :, :, 3] = 1.0
        vvm = np.ascontiguousarray(vvm.reshape(NV2, P, 32)).astype(FP8)
        per_core.append({
            "VVM": vvm,
            "VFLAT": np.ascontiguousarray(vb.reshape(BL, FLAT)),
            "IMG": np.ascontiguousarray(img[c * BL:(c + 1) * BL]).astype(BF16),
        })
    return shared, per_core


def _build_program(has_bias1, has_bias2, has_bias3, has_fcb):
    """Emit the Bass/Tile program (identical on all cores)."""
    from concourse import bacc, bass, mybir, tile
    from concourse.masks import make_identity

    f32 = mybir.dt.float32
    bf16 = mybir.dt.bfloat16
    fp8 = mybir.dt.float8e4
    AF = mybir.ActivationFunctionType
    DR = mybir.MatmulPerfMode.DoubleRow

    nc = bacc.Bacc(trn_type="TRN2")

    d_a2t = nc.dram_tensor("A2T", [NV2, P, 2 * V], fp8, kind="ExternalInput")
    d_w12a = nc.dram_tensor("W12A", [3, F], bf16, kind="ExternalInput")
    d_w12a16 = nc.dram_tensor("W12A16DR", [BL, 8, 2 * F], fp8, kind="ExternalInput")
    d_biasp1 = nc.dram_tensor("BIASP1", [2, F], bf16, kind="ExternalInput")
    d_rho2 = nc.dram_tensor("RHO2", [1, V], bf16, kind="ExternalInput")
    d_rho1 = nc.dram_tensor("RHO1", [2, V], bf16, kind="ExternalInput")
    d_w12b = nc.dram_tensor("W12B", [P, 4 * F], bf16, kind="ExternalInput")
    d_w34 = nc.dram_tensor("W34", [P, 4 * F], fp8, kind="ExternalInput")
    d_w56 = nc.dram_tensor("W56", [P, 64], fp8, kind="ExternalInput")
    d_biasp2 = nc.dram_tensor("BIASP2", [2, F], bf16, kind="ExternalInput")
    d_biasp3 = nc.dram_tensor("BIASP3", [2, BL * 3], bf16, kind="ExternalInput")
    d_fcw1 = nc.dram_tensor("FCW1", [NK1DR, P, 2 * FC_H], fp8, kind="ExternalInput")
    d_fcw2 = nc.dram_tensor("FCW2", [NK2DR, P, 2 * FC_H], fp8, kind="ExternalInput")
    d_fcw3 = nc.dram_tensor("FCW3", [24, P, 2 * FC_H], fp8, kind="ExternalInput")
    d_fcb1 = nc.dram_tensor("FCB1", [BL, FC_H], f32, kind="ExternalInput")
    d_fcb2 = nc.dram_tensor("FCB2", [BL, FC_H], f32, kind="ExternalInput")
    d_fcb3 = nc.dram_tensor("FCB3", [BL, FLAT], f32, kind="ExternalInput")
    d_vvm = nc.dram_tensor("VVM", [NV2, P, 32], fp8, kind="ExternalInput")
    d_vflat = nc.dram_tensor("VFLAT", [BL, FLAT], f32, kind="ExternalInput")
    d_img = nc.dram_tensor("IMG", [BL, IMG_F], bf16, kind="ExternalInput")
    d_out = nc.dram_tensor("OUT", [BL, FLAT], f32, kind="ExternalOutput")

    G = BL * 3  # 12: per-vertex-chunk group width (batch x coord)
    KX1 = 6 if has_bias1 else 4  # x1 folded-matmul contraction depth

    with tile.TileContext(nc) as tc:
        with (
            tc.tile_pool(name="const", bufs=1) as const_pool,
            tc.tile_pool(name="x", bufs=2) as x_pool,
            tc.tile_pool(name="tbf", bufs=1) as tbf_pool,
            tc.tile_pool(name="work", bufs=1) as work_pool,
            tc.tile_pool(name="t3p", bufs=2) as t3_pool,
            tc.tile_pool(name="stream", bufs=32) as stream_pool,
            tc.tile_pool(name="hfin", bufs=2) as hfin_pool,
            tc.tile_pool(name="psA", bufs=3, space="PSUM") as psA,
            tc.tile_pool(name="psB", bufs=2, space="PSUM") as psB,
        ):
            # ---------- resident constants ----------
            # vvm first on the sync queue (32KB, needed by phase0's first
            # LDW at ~7us), then the 4.2MB A2T stream that paces phase0.
            vvm = const_pool.tile([P, NV2 * 32], fp8, tag="vvm")
            for uc2 in range(NV2):
                nc.sync.dma_start(
                    out=vvm[:, uc2 * 32:(uc2 + 1) * 32], in_=d_vvm[uc2]
                )
            a2t = []
            for uc2 in range(NV2):
                t = const_pool.tile([P, 2 * V], fp8, tag=f"a2t{uc2}")
                nc.sync.dma_start(out=t[:], in_=d_a2t[uc2])
                a2t.append(t)

            # identities first: make_identity runs on the gpsimd engine and
            # must precede the gpsimd DMA triggers below, which would
            # otherwise delay the PE warm-up transpose by ~10us.
            ident = const_pool.tile([P, P], f32, tag="ident")
            make_identity(nc, ident[:])
            ident_bf = const_pool.tile([P, P], bf16, tag="ident_bf")
            make_identity(nc, ident_bf[:])

            # gpsimd DMA queue, in need-order: w12b (c1, ~15us), w34
            # (t2 of batch 0, ~30us), then the small per-batch operands.
            # The sync queue carries ONLY vvm + the A2T stream (+ output
            # stores), so A2T runs at full HBM bandwidth.
            w12b = const_pool.tile([P, 4 * F], bf16, tag="w12b")
            nc.gpsimd.dma_start(out=w12b[:], in_=d_w12b[:])
            w34 = const_pool.tile([P, 4 * F], fp8, tag="w34")
            nc.gpsimd.dma_start(out=w34[:], in_=d_w34[:])

            # dummy transpose: absorbs the gpsimd(identity) wait on the PE
            # clock -- walrus allows only ONE sync wait on transpose-mode
            # matmuls (S3 LW struct), so later transposes must carry only
            # their data dependency.
            ps_warm = psA.tile([1, P], f32, tag="psA")
            nc.tensor.transpose(
                out=ps_warm[:], in_=ident[:, 0:1], identity=ident[:]
            )
            vflat = const_pool.tile([BL, FLAT], f32, tag="vflat")
            nc.vector.tensor_copy(out=vflat[0:1, 0:P], in_=ps_warm[:])
            # HAM warm-up: dummy matmuls on the identity while the A2T
            # tiles stream in; keeps the PE activity monitor at K=8/8 so
            # the real aggregation starts at 2.4 GHz instead of 1.2.
            ps_w2 = psB.tile([P, F], f32, tag="psB")

            def ham_keepalive(n):
                for _ in range(n):
                    nc.tensor.matmul(
                        out=ps_w2[:, :P],
                        lhsT=ident_bf[:],
                        rhs=ident_bf[:],
                        start=True,
                        stop=True,
                    )

            ham_keepalive(40)
            nc.vector.tensor_copy(out=vflat[0:1, 0:P], in_=ps_w2[:1, :P])

            # x1 operands.  No-bias path: phase0 emits [16, V] batch
            # groups [av_b(3); rho2] directly (ones column in vvm), and x1
            # contracts over all 16 rows with a per-batch lhsT whose other
            # batches' rows are zero -- no post-phase0 gather DMAs at all.
            # Bias path (unused in the shipped model): per-batch [6, V]
            # rhs assembly as before.
            img_all = const_pool.tile([P, NF * BL], bf16, tag="img_all")
            for b in range(BL):
                nc.gpsimd.dma_start(
                    out=img_all[:].rearrange("p (k b) -> p k b", k=NF)[:, :, b],
                    in_=d_img[b].rearrange("(k p) -> p k", p=P),
                )
            av4 = []
            lhsT4 = []
            lhsT16 = []
            for b in range(BL):
                if has_bias1:
                    avb = const_pool.tile([KX1, V], bf16, tag=f"av4_{b}")
                    nc.gpsimd.dma_start(out=avb[3:4, :], in_=d_rho2[:])
                    nc.gpsimd.dma_start(out=avb[4:6, :], in_=d_rho1[:])
                    av4.append(avb)
                    lb = const_pool.tile([KX1, F], bf16, tag=f"lhsT4_{b}")
                    nc.gpsimd.dma_start(out=lb[0:3, :], in_=d_w12a[:])
                    nc.gpsimd.dma_start(out=lb[4:6, :], in_=d_biasp1[:])
                    lhsT4.append(lb)
                else:
                    lb = const_pool.tile([8, 2 * F], fp8, tag=f"lhsT16_{b}")
                    nc.gpsimd.dma_start(out=lb[:], in_=d_w12a16[b])
                    lhsT16.append(lb)
            w56 = const_pool.tile([P, 64], fp8, tag="w56")
            nc.gpsimd.dma_start(out=w56[:], in_=d_w56[:])
            if has_bias2 or has_bias3:
                rho1 = const_pool.tile([2, V], bf16, tag="rho1")
                nc.gpsimd.dma_start(out=rho1[:], in_=d_rho1[:])
            if has_bias2:
                biasp2 = const_pool.tile([2, F], bf16, tag="biasp2")
                nc.gpsimd.dma_start(out=biasp2[:], in_=d_biasp2[:])
            if has_bias3:
                biasp3 = const_pool.tile([2, BL * 3], bf16, tag="biasp3")
                nc.gpsimd.dma_start(out=biasp3[:], in_=d_biasp3[:])

            def emit_c1():
                # c1[b] = img_b @ W12b -> [BL, 512], emitted between the
                # two phase0 halves (needs only img+w12b, which land ~8us;
                # must not gate phase0's DMA-paced h=0 pass).
                img3 = img_all[:].rearrange("p (k b) -> p k b", k=NF)
                ps_c1 = psB.tile([BL, F], f32, tag="psB")
                for kc in range(NF):
                    nc.tensor.matmul(
                        out=ps_c1[:],
                        lhsT=img3[:, kc],
                        rhs=w12b[:, kc * F:(kc + 1) * F],
                        start=(kc == 0),
                        stop=(kc == NF - 1),
                    )
                c1_all = work_pool.tile(
                    [BL, F], bf16 if has_bias1 else fp8, tag="c1"
                )
                nc.vector.tensor_copy(out=c1_all[:], in_=ps_c1[:])
                for b in range(BL):
                    # no-bias: c1 slot is g=b*4+3 -> p=g%8, j=g//8
                    g = 4 * b + 3
                    dst = (lhsT4[b][3:4, :] if has_bias1
                           else lhsT16[b][g % 8:g % 8 + 1,
                                          (g // 8) * F:(g // 8 + 1) * F])
                    nc.scalar.dma_start(out=dst, in_=c1_all[b:b + 1, :])

            # ---------- phase 0: verts aggregation, feature-major ----------
            # av^T[(b,cc), v] = sum_u verts[u,(b,cc)] * A2T[u, v] : lhsT = vvm
            # chunks (stationary, tiny), rhs = A2T rows (N=512 streams).
            # Wide-N streaming; also lets PE start as soon as a2t[0] lands.
            avt_bf = const_pool.tile([16, V], bf16 if has_bias1 else fp8,
                                     tag="avt")
            if not has_bias1:
                # x1's DoubleRow rhs: row g=j*8+p at partition p, half j.
                avt_dr = const_pool.tile([8, 2 * V], fp8, tag="avtdr")
                avt_dr3 = avt_dr[:].rearrange("p (j v) -> p j v", j=2)
            for h in range(2):
                ps = psA.tile([16, 1024], f32, tag="psA")
                for uc2 in range(NV2):
                    lhsT = vvm[:, uc2 * 32:(uc2 + 1) * 32].rearrange(
                        "p (j g) -> p j g", j=2
                    )
                    rhs3 = a2t[uc2][:].rearrange("p (j v) -> p j v", j=2)
                    for n2 in range(2):
                        col = h * 1024 + n2 * 512
                        nc.tensor.matmul(
                            out=ps[:, n2 * 512:(n2 + 1) * 512],
                            lhsT=lhsT,
                            rhs=rhs3[:, :, col:col + 512],
                            start=(uc2 == 0),
                            stop=(uc2 == NV2 - 1),
                            perf_mode=DR,
                        )
                nc.vector.tensor_copy(
                    out=avt_bf[:, h * 1024:(h + 1) * 1024], in_=ps[:]
                )
                if not has_bias1:
                    for j in range(2):
                        nc.scalar.dma_start(
                            out=avt_dr3[:, j, h * 1024:(h + 1) * 1024],
                            in_=avt_bf[j * 8:j * 8 + 8,
                                       h * 1024:(h + 1) * 1024],
                        )
                if h == 0:
                    emit_c1()
                if has_bias1:
                    # per-batch row triples -> rows 0-2 of the per-batch
                    # rhs tiles (SBUF->SBUF DMA: compute engines cannot
                    # address partition offsets not in {0,32,64,96})
                    for b in range(BL):
                        nc.scalar.dma_start(
                            out=av4[b][0:3, h * 1024:(h + 1) * 1024],
                            in_=avt_bf[b * 4:b * 4 + 3,
                                       h * 1024:(h + 1) * 1024],
                        )

            # verts for the output add, loaded early on the gpsimd queue
            # (before the fcw streams that occupy it for the rest of the
            # kernel).
            nc.gpsimd.dma_start(out=vflat[:], in_=d_vflat[:])

            # t3 storage across batches, feature-major [(b,cc), v] f32
            t3t_all = const_pool.tile([G, V], bf16, tag="t3t_all")

            w56_3d = w56[:].rearrange("p (k j c) -> p k j c", k=2, j=2)  # c slot = 16
            w34_3d = w34[:].rearrange("p (k j n) -> p k j n", k=2, j=2)

            # ---------- per batch: pair1 -> pair2 -> t3 ----------
            for b in range(BL):
                # x1 feature-major [f, v] = relu(single k=4/6 matmul folding
                #   verts agg + image rank-1 term [+ biases]), fp8
                x1_all = tbf_pool.tile([P, NF * V], fp8, tag="x1")
                x1_3d = x1_all[:].rearrange("p (f v) -> p f v", f=NF)
                for fc in range(NF):
                    for nh in range(2):
                        ps = psA.tile([P, 1024], f32, tag="psA")
                        col = nh * 1024
                        if has_bias1:
                            for n2 in range(2):
                                nc.tensor.matmul(
                                    out=ps[:, n2 * 512:(n2 + 1) * 512],
                                    lhsT=lhsT4[b][:, fc * P:(fc + 1) * P],
                                    rhs=av4[b][:, col + n2 * 512:
                                               col + (n2 + 1) * 512],
                                    start=True,
                                    stop=True,
                                )
                        else:
                            lhsT_x1 = lhsT16[b][:].rearrange(
                                "p (j f) -> p j f", j=2
                            )[:, :, fc * P:(fc + 1) * P]
                            for n2 in range(2):
                                nc.tensor.matmul(
                                    out=ps[:, n2 * 512:(n2 + 1) * 512],
                                    lhsT=lhsT_x1,
                                    rhs=avt_dr3[:, :, col + n2 * 512:
                                                col + (n2 + 1) * 512],
                                    start=True,
                                    stop=True,
                                    perf_mode=DR,
                                )
                        if nh == 0:
                            nc.vector.tensor_scalar_max(
                                out=x1_all[:, fc * V + nh * 1024:
                                           fc * V + (nh + 1) * 1024],
                                in0=ps[:],
                                scalar1=0.0,
                            )
                        else:
                            nc.scalar.activation(
                                out=x1_all[:, fc * V + nh * 1024:
                                           fc * V + (nh + 1) * 1024],
                                in_=ps[:],
                                func=AF.Relu,
                            )

                # t2 vertex-major fp8 [v, f] via DoubleRow over k=f
                t2_f8 = tbf_pool.tile([P, NV * F], fp8, tag="t2")
                for vc in range(NV):
                    ps = psB.tile([P, F], f32, tag="psB")
                    for fc2 in range(2):
                        nc.tensor.matmul(
                            out=ps[:],
                            lhsT=x1_3d[:, fc2 * 2:fc2 * 2 + 2,
                                       vc * P:(vc + 1) * P],
                            rhs=w34_3d[:, fc2],
                            start=(fc2 == 0),
                            stop=(fc2 == 1),
                            perf_mode=DR,
                        )
                    if vc % 2 == 0:
                        nc.vector.tensor_copy(
                            out=t2_f8[:, vc * F:(vc + 1) * F], in_=ps[:]
                        )
                    else:
                        nc.scalar.copy(
                            out=t2_f8[:, vc * F:(vc + 1) * F], in_=ps[:]
                        )

                # z2 = A^2 t2 (feature-major out, DoubleRow) ; x2 = relu,
                # stored fp8 as DoubleRow k-pairs for the t3 transform.
                t2_3d = t2_f8[:].rearrange("p (u j f) -> p u j f", u=NV2, j=2)
                x2p = [x_pool.tile([P, 2 * V], fp8, tag="x", name=f"x2p{i}")
                       for i in range(2)]
                for fc in range(NF):
                    xoff = (fc % 2) * V
                    xt = x2p[fc // 2]
                    for nh in range(2):
                        ps = psA.tile([P, 1024], f32, tag="psA")
                        for uc2 in range(NV2):
                            lhsT = t2_3d[:, uc2, :, fc * P:(fc + 1) * P]
                            rhs3 = a2t[uc2][:].rearrange(
                                "p (j v) -> p j v", j=2
                            )
                            for n2 in range(2):
                                col = nh * 1024 + n2 * 512
                                nc.tensor.matmul(
                                    out=ps[:, n2 * 512:(n2 + 1) * 512],
                                    lhsT=lhsT,
                                    rhs=rhs3[:, :, col:col + 512],
                                    start=(uc2 == 0),
                                    stop=(uc2 == NV2 - 1 and not has_bias2),
                                    perf_mode=DR,
                                )
                        if has_bias2:
                            for n2 in range(2):
                                col = nh * 1024 + n2 * 512
                                nc.tensor.matmul(
                                    out=ps[:, n2 * 512:(n2 + 1) * 512],
                                    lhsT=biasp2[:, fc * P:(fc + 1) * P],
                                    rhs=rho1[:, col:col + 512],
                                    start=False,
                                    stop=True,
                                )
                        if nh == 0:
                            nc.vector.tensor_scalar_max(
                                out=xt[:, xoff + nh * 1024:
                                       xoff + (nh + 1) * 1024],
                                in0=ps[:],
                                scalar1=0.0,
                            )
                        else:
                            nc.scalar.activation(
                                out=xt[:, xoff + nh * 1024:
                                       xoff + (nh + 1) * 1024],
                                in_=ps[:],
                                func=AF.Relu,
                            )

                # t3^T = (x2 @ W56)^T feature-major: lhsT = W56 DR chunks
                # (stationary [128,2,3]), rhs = x2 fp8 k-pairs (N=512)
                t3t_b = t3_pool.tile([3, V], bf16, tag="t3t")
                for h in range(2):
                    ps = psA.tile([3, 1024], f32, tag="psA")
                    for kc2 in range(2):
                        rhs3 = x2p[kc2][:].rearrange("p (j v) -> p j v", j=2)
                        for n2 in range(2):
                            col = h * 1024 + n2 * 512
                            nc.tensor.matmul(
                                out=ps[:, n2 * 512:(n2 + 1) * 512],
                                lhsT=w56_3d[:, kc2, :, :3],
                                rhs=rhs3[:, :, col:col + 512],
                                start=(kc2 == 0),
                                stop=(kc2 == 1),
                                perf_mode=DR,
                            )
                    nc.vector.tensor_copy(
                        out=t3t_b[:, h * 1024:(h + 1) * 1024], in_=ps[:]
                    )
                    nc.scalar.dma_start(
                        out=t3t_all[b * 3:(b + 1) * 3,
                                    h * 1024:(h + 1) * 1024],
                        in_=t3t_b[:, h * 1024:(h + 1) * 1024],
                    )

            # ---------- pair3 aggregation, all batches ----------
            # transpose t3^T -> vertex-major fp8 tiles [128, (b,cc)]
            t3_bf = const_pool.tile([P, NV2 * 32], fp8, tag="t3bf")
            for vc in range(NV):
                ps = psA.tile([P, G], bf16, tag="psA")
                nc.tensor.transpose(
                    out=ps[:],
                    in_=t3t_all[:, vc * P:(vc + 1) * P],
                    identity=ident_bf[:G, :G],
                )
                nc.vector.tensor_copy(
                    out=t3_bf[:, vc * 16:vc * 16 + G],
                    in_=ps[:],
                )
            # feature-major agg: z3^T[(b,cc), v] ; then relu
            x3t = const_pool.tile([G, V], bf16, tag="x3t")
            for h in range(2):
                ps = psA.tile([G, 1024], f32, tag="psA")
                t3_3d = t3_bf[:].rearrange("p (u j g) -> p u j g",
                                            u=NV2, j=2, g=16)
                for uc2 in range(NV2):
                    rhs3 = a2t[uc2][:].rearrange("p (j v) -> p j v", j=2)
                    for n2 in range(2):
                        col = h * 1024 + n2 * 512
                        nc.tensor.matmul(
                            out=ps[:, n2 * 512:(n2 + 1) * 512],
                            lhsT=t3_3d[:, uc2, :, :G],
                            rhs=rhs3[:, :, col:col + 512],
                            start=(uc2 == 0),
                            stop=(uc2 == NV2 - 1 and not has_bias3),
                            perf_mode=DR,
                        )
                if has_bias3:
                    for n2 in range(2):
                        col = h * 1024 + n2 * 512
                        nc.tensor.matmul(
                            out=ps[:, n2 * 512:(n2 + 1) * 512],
                            lhsT=biasp3[:],
                            rhs=rho1[:, col:col + 512],
                            start=False,
                            stop=True,
                        )
                nc.vector.tensor_scalar_max(
                    out=x3t[:, h * 1024:(h + 1) * 1024],
                    in0=ps[:],
                    scalar1=0.0,
                )
            # transpose back to vertex-major with (b,cc)->(cc,b) permute;
            # fp8: this is FC1's DoubleRow lhsT ([128, (kc2, j, b)] layout
            # == [128, (kc, b)] since the slot offset is kc*BL either way).
            x3_f8 = const_pool.tile([P, NV * 48], fp8, tag="x3f8")
            for dc in range(NV):
                ps = psA.tile([P, G], bf16, tag="psA")
                nc.tensor.transpose(
                    out=ps[:],
                    in_=x3t[:, dc * P:(dc + 1) * P],
                    identity=ident_bf[:G, :G],
                )
                nc.vector.tensor_copy(
                    out=x3_f8[:, dc * 48:(dc + 1) * 48]
                    .rearrange("p (c s) -> p c s", s=16)[:, :, :BL],
                    in_=ps[:].rearrange("p (b c) -> p c b", c=3),
                )

            # ---------- FC head (all batches together, fp8 DoubleRow) ----
            x3v = x3_f8[:].rearrange("p (k j s) -> p k j s", k=NK1DR, j=2)
            ps_h1 = psA.tile([BL, FC_H], f32, tag="psA")
            for kc2 in range(NK1DR):
                wt = stream_pool.tile([P, 2 * FC_H], fp8, tag="fcw")
                nc.gpsimd.dma_start(out=wt[:], in_=d_fcw1[kc2])
                wv = wt[:].rearrange("p (j n) -> p j n", j=2)
                for n2 in range(2):
                    nc.tensor.matmul(
                        out=ps_h1[:, n2 * 512:(n2 + 1) * 512],
                        lhsT=x3v[:, kc2, :, :BL],
                        rhs=wv[:, :, n2 * 512:(n2 + 1) * 512],
                        start=(kc2 == 0),
                        stop=(kc2 == NK1DR - 1),
                        perf_mode=DR,
                    )
            h1 = hfin_pool.tile([BL, FC_H], bf16, tag="hfin")
            if has_fcb:
                fcb1_sb = hfin_pool.tile([BL, FC_H], f32, tag="fcb")
                nc.sync.dma_start(out=fcb1_sb[:], in_=d_fcb1[:])
                nc.vector.tensor_add(out=h1[:], in0=ps_h1[:], in1=fcb1_sb[:])
            else:
                nc.vector.tensor_copy(out=h1[:], in_=ps_h1[:])

            # transpose h1 -> h1T fp8 [128, (kc, b)] (FC2 DoubleRow lhsT);
            # bf16 transposes run single-pass (fp32 is two-pass LOW_HIGH).
            h1T = const_pool.tile([P, NKFC2 * 16], fp8, tag="h1T")
            for kc in range(NKFC2):
                ps = psA.tile([P, BL], bf16, tag="psA")
                nc.tensor.transpose(
                    out=ps[:],
                    in_=h1[:, kc * P:(kc + 1) * P],
                    identity=ident_bf[:BL, :BL],
                )
                nc.vector.tensor_copy(
                    out=h1T[:, kc * 16:kc * 16 + BL], in_=ps[:]
                )

            h1v = h1T[:].rearrange("p (k j s) -> p k j s", k=NK2DR, j=2)
            ps_h2 = psA.tile([BL, FC_H], f32, tag="psA")
            for kc2 in range(NK2DR):
                wt = stream_pool.tile([P, 2 * FC_H], fp8, tag="fcw")
                nc.gpsimd.dma_start(out=wt[:], in_=d_fcw2[kc2])
                wv = wt[:].rearrange("p (j n) -> p j n", j=2)
                for n2 in range(2):
                    nc.tensor.matmul(
                        out=ps_h2[:, n2 * 512:(n2 + 1) * 512],
                        lhsT=h1v[:, kc2, :, :BL],
                        rhs=wv[:, :, n2 * 512:(n2 + 1) * 512],
                        start=(kc2 == 0),
                        stop=(kc2 == NK2DR - 1),
                        perf_mode=DR,
                    )
            h2 = hfin_pool.tile([BL, FC_H], bf16, tag="hfin")
            if has_fcb:
                fcb2_sb = hfin_pool.tile([BL, FC_H], f32, tag="fcb")
                nc.sync.dma_start(out=fcb2_sb[:], in_=d_fcb2[:])
                nc.vector.tensor_add(out=h2[:], in0=ps_h2[:], in1=fcb2_sb[:])
            else:
                nc.vector.tensor_copy(out=h2[:], in_=ps_h2[:])

            h2T = const_pool.tile([P, NKFC2 * 16], fp8, tag="h2T")
            for kc in range(NKFC2):
                ps = psA.tile([P, BL], bf16, tag="psA")
                nc.tensor.transpose(
                    out=ps[:],
                    in_=h2[:, kc * P:(kc + 1) * P],
                    identity=ident_bf[:BL, :BL],
                )
                nc.vector.tensor_copy(
                    out=h2T[:, kc * 16:kc * 16 + BL], in_=ps[:]
                )

            # FC3, software-pipelined: chunk ch's tanh/store tail is
            # emitted under chunk ch+1's matmuls.  The whole tail stays in
            # batch-major [BL, 1024] layout (OUT is [BL, FLAT]): no
            # transposes, and tanh reads the PSUM accumulator directly.
            h2v = h2T[:].rearrange("p (k j s) -> p k j s", k=NK2DR, j=2)
            NCH = FLAT // FC_H  # 6
            ps_acc = [None] * NCH

            def fc3_tail(ch):
                cols = slice(ch * FC_H, (ch + 1) * FC_H)
                dch = hfin_pool.tile([BL, FC_H], f32, tag="dch",
                                     name=f"dch{ch}")
                if has_fcb:
                    fcb3_sb = hfin_pool.tile([BL, FC_H], f32, tag="fcb",
                                             name=f"fcb3_{ch}")
                    nc.gpsimd.dma_start(
                        out=fcb3_sb[:],
                        in_=d_fcb3[:, ch * FC_H:(ch + 1) * FC_H],
                    )
                    h3sb = hfin_pool.tile([BL, FC_H], f32, tag="hfin",
                                          name=f"h3sb{ch}")
                    nc.vector.tensor_add(
                        out=h3sb[:], in0=ps_acc[ch][:], in1=fcb3_sb[:]
                    )
                    nc.scalar.activation(out=dch[:], in_=h3sb[:],
                                         func=AF.Tanh)
                else:
                    nc.scalar.activation(out=dch[:], in_=ps_acc[ch][:],
                                         func=AF.Tanh)
                och = hfin_pool.tile([BL, FC_H], f32, tag="och",
                                     name=f"och{ch}")
                nc.vector.tensor_scalar_mul(
                    out=och[:], in0=dch[:], scalar1=0.1
                )
                nc.vector.tensor_add(
                    out=och[:], in0=och[:], in1=vflat[:, cols]
                )
                nc.sync.dma_start(out=d_out[:, cols], in_=och[:])

            for ch in range(NCH):
                ps = psA.tile([BL, FC_H], f32, tag="psA",
                              name=f"ps_fc3_{ch}")
                ps_acc[ch] = ps
                for kc2 in range(NK2DR):
                    wt = stream_pool.tile([P, 2 * FC_H], fp8, tag="fcw")
                    nc.gpsimd.dma_start(out=wt[:], in_=d_fcw3[ch * NK2DR + kc2])
                    wv = wt[:].rearrange("p (j n) -> p j n", j=2)
                    for n2 in range(2):
                        nc.tensor.matmul(
                            out=ps[:, n2 * 512:(n2 + 1) * 512],
                            lhsT=h2v[:, kc2, :, :BL],
                            rhs=wv[:, :, n2 * 512:(n2 + 1) * 512],
                            start=(kc2 == 0),
                            stop=(kc2 == NK2DR - 1),
                            perf_mode=DR,
                        )
                if ch >= 1:
                    fc3_tail(ch - 1)
            fc3_tail(NCH - 1)

    nc.finalize()
    return nc


def build_in_maps(inputs):
    """Host prep + per-core input maps (exposed for testing)."""
    shared, per_core = _host_prep(inputs)
    key = (shared["HAS_BIAS1"], shared["HAS_BIAS2"], shared["HAS_BIAS3"],
           shared["HAS_FCB"])
    shared_arrays = {k: v for k, v in shared.items() if isinstance(v, np.ndarray)}
    in_maps = []
    for c in range(N_CORES):
        m = dict(shared_arrays)
        m.update(per_core[c])
        in_maps.append(m)
    return key, in_maps


def unpack_out(raw):
    return np.asarray(raw, np.float32).reshape(BL, V, 3)


def kernel(**inputs):
    key, in_maps = build_in_maps(inputs)
    if key not in _CACHE:
        _CACHE[key] = _build_program(*key)
    nc = _CACHE[key]

    from concourse.bass_utils import run_bass_kernel_spmd

    res = run_bass_kernel_spmd(nc, in_maps, list(range(N_CORES)))
    out = np.empty((B, V, 3), np.float32)
    for c in range(N_CORES):
        out[c * BL:(c + 1) * BL] = unpack_out(res.results[c]["OUT"])
    return out


# revision 26
# speedup vs baseline: 1.0192x; 1.0192x over previous
"""Trainium2 Bass kernel for nn_GCNModel (6-layer GCN + 3-layer FC mesh deformer).

Strategy
--------
Data-parallel over batch B=32 across 8 NeuronCores (4 batch elements each).

Algebraic restructuring (host side, exact):
  ReLU only follows GCN layers 2, 4, 6, so each pair of GCN layers collapses:
      A(A x W1 + 1 b1^T) W2 + 1 b2^T
        = A^2 x (W1 W2) + (A 1) (b1 W2)^T + 1 b2^T
  with A the dense-ified normalized adjacency.  Three aggregations with a
  host-precomputed dense A^2 replace six sparse gather/scatter aggregations.
  Further:
    * pair 1's aggregation input is rank-3 (x = [verts | 1 img^T]):
      A^2 x W12 = (A^2 verts) W12[:3] + (A^2 1) (img W12[3:])^T
      so the wide aggregation reduces to a width-3 one plus rank-1 terms,
      all folded into ONE k=4 (k=6 with biases) matmul per output tile.
    * pair 3 aggregates after the [512,3] transform (width 3).
  Only pair 2 needs a full width-512 dense A^2 apply per batch element.

Everything on the critical path runs in fp8 (e4m3) DoubleRow matmuls with
fp32 PSUM accumulation where the layout permits: the A^2 aggregations, the
W34 transform, the W56 transform, and the whole FC head (weights, x3, h1,
h2 all fp8).  Host-validated vs the fp32 reference: ~6e-3 max relative
error (the output is dominated by `vertices` plus a 0.1-scaled
tanh-squashed deformation).

Layouts alternate vertex-major / feature-major so no transposes are needed
in the hot path:
  agg (contracts over vertices):  lhsT = t (vertex-major), rhs = A2T rows
                                  -> feature-major output
  transform (contracts over features): lhsT = x (feature-major), rhs = W
                                  -> vertex-major output
"""

import numpy as np
import ml_dtypes

B, V, E, IMG_F = 32, 2048, 12288, 512
N_CORES = 8
BL = B // N_CORES  # 4 batch elements per core
P = 128
NV = V // P   # 16 vertex chunks
F = 512
NF = F // P   # 4 feature chunks
FC_H = 1024
FLAT = V * 3  # 6144
NKFC1 = FLAT // P  # 48
NKFC2 = FC_H // P  # 8
NV2 = NV // 2  # 8 double-row vertex chunks
NK1DR = NKFC1 // 2  # 24 DoubleRow k-tiles for FC1
NK2DR = NKFC2 // 2  # 4 DoubleRow k-tiles for FC2/FC3

BF16 = ml_dtypes.bfloat16
FP8 = ml_dtypes.float8_e4m3

_CACHE = {}


def _host_prep(inputs):
    """Exact (fp64) host-side algebra: dense A^2, collapsed weights, shards."""
    ei = np.asarray(inputs["edge_index"])
    src = np.concatenate([ei[0], np.arange(V)]).astype(np.int64)
    dst = np.concatenate([ei[1], np.arange(V)]).astype(np.int64)
    deg = np.zeros(V)
    np.add.at(deg, dst, 1.0)
    dinv = 1.0 / np.sqrt(deg)
    normv = dinv[src] * dinv[dst]
    A = np.zeros((V, V))
    np.add.at(A, (dst, src), normv)
    A2 = A @ A
    rho = (A @ np.ones(V)).astype(np.float32)
    rho2 = (A2 @ np.ones(V)).astype(np.float32)

    W = [np.asarray(inputs[f"W{i}"], np.float64) for i in range(1, 7)]
    bb = [np.asarray(inputs[f"b{i}"], np.float64) for i in range(1, 7)]
    W12 = W[0] @ W[1]
    W34 = W[2] @ W[3]
    W56 = W[4] @ W[5]
    bias1 = bb[0] @ W[1]  # pairs with rho
    bias2 = bb[2] @ W[3]
    bias3 = bb[4] @ W[5]
    b2, b4, b6 = bb[1], bb[3], bb[5]

    shared = {}
    # A2T in fp8 DoubleRow layout: [uc2][p, j*V + v] = A2T[uc2*256+j*128+p, v]
    A2T = np.ascontiguousarray(A2.T).astype(np.float32)
    shared["A2T"] = np.ascontiguousarray(
        A2T.reshape(NV2, 2, P, V).transpose(0, 2, 1, 3).reshape(NV2, P, 2 * V)
    ).astype(FP8)
    # k=3 static lhsT rows for pair1 (verts); c1 (img term) is folded in as
    # lhsT row 3 (rhs row 3 = rho2), biases as rows 4-5 (rhs rows = rho1).
    shared["W12A"] = np.asarray(W12[:3], np.float32).astype(BF16)
    bias_pack1 = np.stack([bias1, b2]).astype(np.float32)  # pairs with rho1
    shared["HAS_BIAS1"] = bool(np.any(bias_pack1))
    shared["BIASP1"] = bias_pack1.astype(BF16)
    shared["RHO2"] = rho2.reshape(1, V).astype(BF16)
    shared["RHO1"] = np.stack([rho, np.ones(V, np.float32)]).astype(BF16)

    def pack_rows(w, ncol):
        # [nk*128, ncol] -> [128, nk*ncol] with chunk kc at cols [kc*ncol:...]
        w = np.asarray(w, np.float32)
        nk = w.shape[0] // P
        return np.ascontiguousarray(
            w.reshape(nk, P, ncol).transpose(1, 0, 2).reshape(P, nk * ncol)
        )

    def pack_dr(w, ncol):
        # [nk2*256, ncol] -> [nk2, 128, 2*ncol]: tile i, row p, col j*ncol+n
        # = w[i*256 + j*128 + p, n]  (DoubleRow k-pair layout)
        w = np.asarray(w, np.float32)
        nk2 = w.shape[0] // 256
        return np.ascontiguousarray(
            w.reshape(nk2, 2, P, ncol).transpose(0, 2, 1, 3)
            .reshape(nk2, P, 2 * ncol)
        )

    shared["W12B"] = pack_rows(W12[3:], F).astype(BF16)
    # W34 in fp8 DoubleRow layout: [p, (fc2, j, fout)] = W34[fc2*256+j*128+p, f]
    W34f = np.asarray(W34, np.float32)
    shared["W34"] = np.ascontiguousarray(
        W34f.reshape(2, 2, P, F).transpose(2, 0, 1, 3).reshape(P, 4 * F)
    ).astype(FP8)
    # W56 fp8 DoubleRow: [p, (kc2, j, c-slot16)] = W56[kc2*256+j*128+p, c]
    # (c slot padded 3->16: dual-fp8 LDW requires 16B-aligned j-stride)
    w56r = np.zeros((2, 2, P, 16), np.float32)
    w56r[:, :, :, :3] = np.asarray(W56, np.float32).reshape(2, 2, P, 3)
    shared["W56"] = np.ascontiguousarray(
        w56r.transpose(2, 0, 1, 3).reshape(P, 64)
    ).astype(FP8)

    # pair2/3 bias packs (zero in the shipped model; matmul-folded if not)
    bias_pack2 = np.stack([bias2, b4]).astype(np.float32)  # [2, 512]
    bias_pack3 = np.zeros((2, BL * 3), np.float32)
    for b in range(BL):
        bias_pack3[0, b * 3:(b + 1) * 3] = bias3
        bias_pack3[1, b * 3:(b + 1) * 3] = b6
    shared["HAS_BIAS2"] = bool(np.any(bias_pack2))
    shared["HAS_BIAS3"] = bool(np.any(bias_pack3))
    shared["BIASP2"] = bias_pack2.astype(BF16)
    shared["BIASP3"] = bias_pack3.astype(BF16)

    # FC weights, fp8 DoubleRow tiles. fcW1 rows permuted: new row
    # (vc*3+c)*128+p corresponds to original row (vc*128+p)*3+c.
    fcW1 = np.asarray(inputs["fcW1"], np.float32)
    idx = (
        (np.arange(NV)[:, None, None] * P + np.arange(P)[None, None, :]) * 3
        + np.arange(3)[None, :, None]
    ).reshape(-1)  # (vc, c, p) -> orig row
    shared["FCW1"] = pack_dr(fcW1[idx], FC_H).astype(FP8)  # [24, 128, 2048]
    shared["FCW2"] = pack_dr(
        np.asarray(inputs["fcW2"], np.float32), FC_H
    ).astype(FP8)  # [4, 128, 2048]
    # FCW3: [(ch*4+kc2), p, j*1024+n] = fcW3[kc2*256+j*128+p, ch*1024+n]
    fcW3 = np.asarray(inputs["fcW3"], np.float32)
    shared["FCW3"] = np.ascontiguousarray(
        fcW3.reshape(NK2DR, 2, P, FLAT // FC_H, FC_H)
        .transpose(3, 0, 2, 1, 4).reshape(24, P, 2 * FC_H)
    ).astype(FP8)
    fcb1 = np.asarray(inputs["fcb1"], np.float32)
    fcb2 = np.asarray(inputs["fcb2"], np.float32)
    fcb3 = np.asarray(inputs["fcb3"], np.float32)
    shared["HAS_FCB"] = bool(np.any(fcb1) or np.any(fcb2) or np.any(fcb3))
    shared["FCB1"] = np.ascontiguousarray(np.broadcast_to(fcb1, (BL, FC_H)))
    shared["FCB2"] = np.ascontiguousarray(np.broadcast_to(fcb2, (BL, FC_H)))
    shared["FCB3"] = np.ascontiguousarray(np.broadcast_to(fcb3, (BL, FLAT)))

    # W12A16DR (no-bias x1 lhsT, fp8 DoubleRow over the 16-row phase0
    # output): [b][p, j*F+f] = w12a16[b][j*8+p, f], with W12A rows at
    # g=b*4+c, zeros elsewhere; the c1 slot g=b*4+3 is filled on device.
    # The j*8+p pairing lets the avt repack DMA read plain partition
    # ranges (avt rows 0-7 -> j=0, rows 8-15 -> j=1).
    w12a16 = np.zeros((BL, 16, F), np.float32)
    for b in range(BL):
        w12a16[b, b * 4:b * 4 + 3, :] = np.asarray(W12[:3], np.float32)
    shared["W12A16DR"] = np.ascontiguousarray(
        w12a16.reshape(BL, 2, 8, F).transpose(0, 2, 1, 3)
        .reshape(BL, 8, 2 * F)
    ).astype(FP8)

    # per-core shards
    verts = np.asarray(inputs["vertices"], np.float32)  # [B, V, 3]
    img = np.asarray(inputs["img_features"], np.float32)  # [B, 512]
    per_core = []
    for c in range(N_CORES):
        vb = verts[c * BL:(c + 1) * BL]  # [BL, V, 3]
        # DoubleRow lhsT: [uc2][p, j*16 + (b*4+cc)] = verts[b, uc2*256+j*128+p, cc]
        # with a ones column at g=b*4+3 so phase0 emits rho2 = A^2 @ 1 as
        # row 3 of every batch group.  (16B-aligned j-stride for dual-fp8.)
        vraw = vb.transpose(1, 0, 2).reshape(NV2, 2, P, BL, 3)
        vvm = np.zeros((NV2, P, 2, BL, 4), np.float32)
        vvm[:, :, :, :, :3] = vraw.transpose(0, 2, 1, 3, 4)
        vvm[:, :, :, :, 3] = 1.0
        vvm = np.ascontiguousarray(vvm.reshape(NV2, P, 32)).astype(FP8)
        per_core.append({
            "VVM": vvm,
            "VFLAT": np.ascontiguousarray(vb.reshape(BL, FLAT)),
            "IMG": np.ascontiguousarray(img[c * BL:(c + 1) * BL]).astype(BF16),
        })
    return shared, per_core


def _build_program(has_bias1, has_bias2, has_bias3, has_fcb):
    """Emit the Bass/Tile program (identical on all cores)."""
    from concourse import bacc, bass, mybir, tile
    from concourse.masks import make_identity

    f32 = mybir.dt.float32
    bf16 = mybir.dt.bfloat16
    fp8 = mybir.dt.float8e4
    AF = mybir.ActivationFunctionType
    DR = mybir.MatmulPerfMode.DoubleRow

    nc = bacc.Bacc(trn_type="TRN2")

    d_a2t = nc.dram_tensor("A2T", [NV2, P, 2 * V], fp8, kind="ExternalInput")
    d_w12a = nc.dram_tensor("W12A", [3, F], bf16, kind="ExternalInput")
    d_w12a16 = nc.dram_tensor("W12A16DR", [BL, 8, 2 * F], fp8, kind="ExternalInput")
    d_biasp1 = nc.dram_tensor("BIASP1", [2, F], bf16, kind="ExternalInput")
    d_rho2 = nc.dram_tensor("RHO2", [1, V], bf16, kind="ExternalInput")
    d_rho1 = nc.dram_tensor("RHO1", [2, V], bf16, kind="ExternalInput")
    d_w12b = nc.dram_tensor("W12B", [P, 4 * F], bf16, kind="ExternalInput")
    d_w34 = nc.dram_tensor("W34", [P, 4 * F], fp8, kind="ExternalInput")
    d_w56 = nc.dram_tensor("W56", [P, 64], fp8, kind="ExternalInput")
    d_biasp2 = nc.dram_tensor("BIASP2", [2, F], bf16, kind="ExternalInput")
    d_biasp3 = nc.dram_tensor("BIASP3", [2, BL * 3], bf16, kind="ExternalInput")
    d_fcw1 = nc.dram_tensor("FCW1", [NK1DR, P, 2 * FC_H], fp8, kind="ExternalInput")
    d_fcw2 = nc.dram_tensor("FCW2", [NK2DR, P, 2 * FC_H], fp8, kind="ExternalInput")
    d_fcw3 = nc.dram_tensor("FCW3", [24, P, 2 * FC_H], fp8, kind="ExternalInput")
    d_fcb1 = nc.dram_tensor("FCB1", [BL, FC_H], f32, kind="ExternalInput")
    d_fcb2 = nc.dram_tensor("FCB2", [BL, FC_H], f32, kind="ExternalInput")
    d_fcb3 = nc.dram_tensor("FCB3", [BL, FLAT], f32, kind="ExternalInput")
    d_vvm = nc.dram_tensor("VVM", [NV2, P, 32], fp8, kind="ExternalInput")
    d_vflat = nc.dram_tensor("VFLAT", [BL, FLAT], f32, kind="ExternalInput")
    d_img = nc.dram_tensor("IMG", [BL, IMG_F], bf16, kind="ExternalInput")
    d_out = nc.dram_tensor("OUT", [BL, FLAT], f32, kind="ExternalOutput")

    G = BL * 3  # 12: per-vertex-chunk group width (batch x coord)
    KX1 = 6 if has_bias1 else 4  # x1 folded-matmul contraction depth

    with tile.TileContext(nc) as tc:
        with (
            tc.tile_pool(name="const", bufs=1) as const_pool,
            tc.tile_pool(name="x", bufs=2) as x_pool,
            tc.tile_pool(name="tbf", bufs=1) as tbf_pool,
            tc.tile_pool(name="work", bufs=1) as work_pool,
            tc.tile_pool(name="t3p", bufs=2) as t3_pool,
            tc.tile_pool(name="stream", bufs=32) as stream_pool,
            tc.tile_pool(name="hfin", bufs=2) as hfin_pool,
            tc.tile_pool(name="psA", bufs=3, space="PSUM") as psA,
            tc.tile_pool(name="psB", bufs=2, space="PSUM") as psB,
        ):
            # ---------- resident constants ----------
            # Everything DMA-critical rides the sync queue in need-order:
            # ~0.9MB of small constants (2.5us), then the 4.2MB A2T stream
            # that paces phase0, then the FC weight streams.  Serializing
            # them on one queue is deliberate: parallel queues would split
            # HBM bandwidth and delay A2T, which gates batch 0's z2.
            w12b = const_pool.tile([P, 4 * F], bf16, tag="w12b")
            nc.sync.dma_start(out=w12b[:], in_=d_w12b[:])
            w34 = const_pool.tile([P, 4 * F], fp8, tag="w34")
            nc.sync.dma_start(out=w34[:], in_=d_w34[:])
            vvm = const_pool.tile([P, NV2 * 32], fp8, tag="vvm")
            for uc2 in range(NV2):
                nc.sync.dma_start(
                    out=vvm[:, uc2 * 32:(uc2 + 1) * 32], in_=d_vvm[uc2]
                )
            a2t = []
            for uc2 in range(NV2):
                t = const_pool.tile([P, 2 * V], fp8, tag=f"a2t{uc2}")
                nc.sync.dma_start(out=t[:], in_=d_a2t[uc2])
                a2t.append(t)

            # identities first: make_identity runs on the gpsimd engine and
            # must precede the gpsimd DMA triggers below, which would
            # otherwise delay the PE warm-up transpose by ~10us.
            ident = const_pool.tile([P, P], f32, tag="ident")
            make_identity(nc, ident[:])
            ident_bf = const_pool.tile([P, P], bf16, tag="ident_bf")
            make_identity(nc, ident_bf[:])

            # remaining small operands on the gpsimd queue (idle
            # otherwise; all fire within the first ~6us).

            # dummy transpose: absorbs the gpsimd(identity) wait on the PE
            # clock -- walrus allows only ONE sync wait on transpose-mode
            # matmuls (S3 LW struct), so later transposes must carry only
            # their data dependency.
            ps_warm = psA.tile([1, P], f32, tag="psA")
            nc.tensor.transpose(
                out=ps_warm[:], in_=ident[:, 0:1], identity=ident[:]
            )
            vflat = const_pool.tile([BL, FLAT], f32, tag="vflat")
            nc.vector.tensor_copy(out=vflat[0:1, 0:P], in_=ps_warm[:])
            # HAM warm-up: dummy matmuls on the identity while the A2T
            # tiles stream in; keeps the PE activity monitor at K=8/8 so
            # the real aggregation starts at 2.4 GHz instead of 1.2.
            ps_w2 = psB.tile([P, F], f32, tag="psB")

            def ham_keepalive(n):
                for _ in range(n):
                    nc.tensor.matmul(
                        out=ps_w2[:, :P],
                        lhsT=ident_bf[:],
                        rhs=ident_bf[:],
                        start=True,
                        stop=True,
                    )

            ham_keepalive(40)
            nc.vector.tensor_copy(out=vflat[0:1, 0:P], in_=ps_w2[:1, :P])

            # x1 operands.  No-bias path: phase0 emits [16, V] batch
            # groups [av_b(3); rho2] directly (ones column in vvm), and x1
            # contracts over all 16 rows with a per-batch lhsT whose other
            # batches' rows are zero -- no post-phase0 gather DMAs at all.
            # Bias path (unused in the shipped model): per-batch [6, V]
            # rhs assembly as before.
            img_all = const_pool.tile([P, NF * BL], bf16, tag="img_all")
            for b in range(BL):
                nc.gpsimd.dma_start(
                    out=img_all[:].rearrange("p (k b) -> p k b", k=NF)[:, :, b],
                    in_=d_img[b].rearrange("(k p) -> p k", p=P),
                )
            av4 = []
            lhsT4 = []
            lhsT16 = []
            for b in range(BL):
                if has_bias1:
                    avb = const_pool.tile([KX1, V], bf16, tag=f"av4_{b}")
                    nc.gpsimd.dma_start(out=avb[3:4, :], in_=d_rho2[:])
                    nc.gpsimd.dma_start(out=avb[4:6, :], in_=d_rho1[:])
                    av4.append(avb)
                    lb = const_pool.tile([KX1, F], bf16, tag=f"lhsT4_{b}")
                    nc.gpsimd.dma_start(out=lb[0:3, :], in_=d_w12a[:])
                    nc.gpsimd.dma_start(out=lb[4:6, :], in_=d_biasp1[:])
                    lhsT4.append(lb)
                else:
                    lb = const_pool.tile([8, 2 * F], fp8, tag=f"lhsT16_{b}")
                    nc.gpsimd.dma_start(out=lb[:], in_=d_w12a16[b])
                    lhsT16.append(lb)
            w56 = const_pool.tile([P, 64], fp8, tag="w56")
            nc.gpsimd.dma_start(out=w56[:], in_=d_w56[:])
            if has_bias2 or has_bias3:
                rho1 = const_pool.tile([2, V], bf16, tag="rho1")
                nc.gpsimd.dma_start(out=rho1[:], in_=d_rho1[:])
            if has_bias2:
                biasp2 = const_pool.tile([2, F], bf16, tag="biasp2")
                nc.gpsimd.dma_start(out=biasp2[:], in_=d_biasp2[:])
            if has_bias3:
                biasp3 = const_pool.tile([2, BL * 3], bf16, tag="biasp3")
                nc.gpsimd.dma_start(out=biasp3[:], in_=d_biasp3[:])

            def emit_c1():
                # c1[b] = img_b @ W12b -> [BL, 512], emitted between the
                # two phase0 halves (needs only img+w12b, which land ~8us;
                # must not gate phase0's DMA-paced h=0 pass).
                img3 = img_all[:].rearrange("p (k b) -> p k b", k=NF)
                ps_c1 = psB.tile([BL, F], f32, tag="psB")
                for kc in range(NF):
                    nc.tensor.matmul(
                        out=ps_c1[:],
                        lhsT=img3[:, kc],
                        rhs=w12b[:, kc * F:(kc + 1) * F],
                        start=(kc == 0),
                        stop=(kc == NF - 1),
                    )
                c1_all = work_pool.tile(
                    [BL, F], bf16 if has_bias1 else fp8, tag="c1"
                )
                nc.vector.tensor_copy(out=c1_all[:], in_=ps_c1[:])
                for b in range(BL):
                    # no-bias: c1 slot is g=b*4+3 -> p=g%8, j=g//8
                    g = 4 * b + 3
                    dst = (lhsT4[b][3:4, :] if has_bias1
                           else lhsT16[b][g % 8:g % 8 + 1,
                                          (g // 8) * F:(g // 8 + 1) * F])
                    nc.scalar.dma_start(out=dst, in_=c1_all[b:b + 1, :])

            # ---------- phase 0: verts aggregation, feature-major ----------
            # av^T[(b,cc), v] = sum_u verts[u,(b,cc)] * A2T[u, v] : lhsT = vvm
            # chunks (stationary, tiny), rhs = A2T rows (N=512 streams).
            # Wide-N streaming; also lets PE start as soon as a2t[0] lands.
            avt_bf = const_pool.tile([16, V], bf16 if has_bias1 else fp8,
                                     tag="avt")
            if not has_bias1:
                # x1's DoubleRow rhs: row g=j*8+p at partition p, half j.
                avt_dr = const_pool.tile([8, 2 * V], fp8, tag="avtdr")
                avt_dr3 = avt_dr[:].rearrange("p (j v) -> p j v", j=2)
            for h in range(2):
                ps = psA.tile([16, 1024], f32, tag="psA")
                for uc2 in range(NV2):
                    lhsT = vvm[:, uc2 * 32:(uc2 + 1) * 32].rearrange(
                        "p (j g) -> p j g", j=2
                    )
                    rhs3 = a2t[uc2][:].rearrange("p (j v) -> p j v", j=2)
                    for n2 in range(2):
                        col = h * 1024 + n2 * 512
                        nc.tensor.matmul(
                            out=ps[:, n2 * 512:(n2 + 1) * 512],
                            lhsT=lhsT,
                            rhs=rhs3[:, :, col:col + 512],
                            start=(uc2 == 0),
                            stop=(uc2 == NV2 - 1),
                            perf_mode=DR,
                        )
                nc.vector.tensor_copy(
                    out=avt_bf[:, h * 1024:(h + 1) * 1024], in_=ps[:]
                )
                if not has_bias1:
                    for j in range(2):
                        nc.scalar.dma_start(
                            out=avt_dr3[:, j, h * 1024:(h + 1) * 1024],
                            in_=avt_bf[j * 8:j * 8 + 8,
                                       h * 1024:(h + 1) * 1024],
                        )
                if h == 0:
                    emit_c1()
                if has_bias1:
                    # per-batch row triples -> rows 0-2 of the per-batch
                    # rhs tiles (SBUF->SBUF DMA: compute engines cannot
                    # address partition offsets not in {0,32,64,96})
                    for b in range(BL):
                        nc.scalar.dma_start(
                            out=av4[b][0:3, h * 1024:(h + 1) * 1024],
                            in_=avt_bf[b * 4:b * 4 + 3,
                                       h * 1024:(h + 1) * 1024],
                        )

            # t3 storage across batches, feature-major [(b,cc), v] f32
            t3t_all = const_pool.tile([G, V], bf16, tag="t3t_all")

            w56_3d = w56[:].rearrange("p (k j c) -> p k j c", k=2, j=2)  # c slot = 16
            w34_3d = w34[:].rearrange("p (k j n) -> p k j n", k=2, j=2)

            # ---------- per batch: pair1 -> pair2 -> t3 ----------
            for b in range(BL):
                # x1 feature-major [f, v] = relu(single k=4/6 matmul folding
                #   verts agg + image rank-1 term [+ biases]), fp8
                x1_all = tbf_pool.tile([P, NF * V], fp8, tag="x1")
                x1_3d = x1_all[:].rearrange("p (f v) -> p f v", f=NF)
                for fc in range(NF):
                    for nh in range(2):
                        ps = psA.tile([P, 1024], f32, tag="psA")
                        col = nh * 1024
                        if has_bias1:
                            for n2 in range(2):
                                nc.tensor.matmul(
                                    out=ps[:, n2 * 512:(n2 + 1) * 512],
                                    lhsT=lhsT4[b][:, fc * P:(fc + 1) * P],
                                    rhs=av4[b][:, col + n2 * 512:
                                               col + (n2 + 1) * 512],
                                    start=True,
                                    stop=True,
                                )
                        else:
                            lhsT_x1 = lhsT16[b][:].rearrange(
                                "p (j f) -> p j f", j=2
                            )[:, :, fc * P:(fc + 1) * P]
                            for n2 in range(2):
                                nc.tensor.matmul(
                                    out=ps[:, n2 * 512:(n2 + 1) * 512],
                                    lhsT=lhsT_x1,
                                    rhs=avt_dr3[:, :, col + n2 * 512:
                                                col + (n2 + 1) * 512],
                                    start=True,
                                    stop=True,
                                    perf_mode=DR,
                                )
                        if nh == 0:
                            nc.vector.tensor_scalar_max(
                                out=x1_all[:, fc * V + nh * 1024:
                                           fc * V + (nh + 1) * 1024],
                                in0=ps[:],
                                scalar1=0.0,
                            )
                        else:
                            nc.scalar.activation(
                                out=x1_all[:, fc * V + nh * 1024:
                                           fc * V + (nh + 1) * 1024],
                                in_=ps[:],
                                func=AF.Relu,
                            )

                # t2 vertex-major fp8 [v, f] via DoubleRow over k=f
                t2_f8 = tbf_pool.tile([P, NV * F], fp8, tag="t2")
                for vc in range(NV):
                    ps = psB.tile([P, F], f32, tag="psB")
                    for fc2 in range(2):
                        nc.tensor.matmul(
                            out=ps[:],
                            lhsT=x1_3d[:, fc2 * 2:fc2 * 2 + 2,
                                       vc * P:(vc + 1) * P],
                            rhs=w34_3d[:, fc2],
                            start=(fc2 == 0),
                            stop=(fc2 == 1),
                            perf_mode=DR,
                        )
                    if vc % 2 == 0:
                        nc.vector.tensor_copy(
                            out=t2_f8[:, vc * F:(vc + 1) * F], in_=ps[:]
                        )
                    else:
                        nc.scalar.copy(
                            out=t2_f8[:, vc * F:(vc + 1) * F], in_=ps[:]
                        )

                # z2 = A^2 t2 (feature-major out, DoubleRow) ; x2 = relu,
                # stored fp8 as DoubleRow k-pairs for the t3 transform.
                t2_3d = t2_f8[:].rearrange("p (u j f) -> p u j f", u=NV2, j=2)
                x2p = [x_pool.tile([P, 2 * V], fp8, tag="x", name=f"x2p{i}")
                       for i in range(2)]
                for fc in range(NF):
                    xoff = (fc % 2) * V
                    xt = x2p[fc // 2]
                    for nh in range(2):
                        ps = psA.tile([P, 1024], f32, tag="psA")
                        for uc2 in range(NV2):
                            lhsT = t2_3d[:, uc2, :, fc * P:(fc + 1) * P]
                            rhs3 = a2t[uc2][:].rearrange(
                                "p (j v) -> p j v", j=2
                            )
                            for n2 in range(2):
                                col = nh * 1024 + n2 * 512
                                nc.tensor.matmul(
                                    out=ps[:, n2 * 512:(n2 + 1) * 512],
                                    lhsT=lhsT,
                                    rhs=rhs3[:, :, col:col + 512],
                                    start=(uc2 == 0),
                                    stop=(uc2 == NV2 - 1 and not has_bias2),
                                    perf_mode=DR,
                                )
                        if has_bias2:
                            for n2 in range(2):
                                col = nh * 1024 + n2 * 512
                                nc.tensor.matmul(
                                    out=ps[:, n2 * 512:(n2 + 1) * 512],
                                    lhsT=biasp2[:, fc * P:(fc + 1) * P],
                                    rhs=rho1[:, col:col + 512],
                                    start=False,
                                    stop=True,
                                )
                        if nh == 0:
                            nc.vector.tensor_scalar_max(
                                out=xt[:, xoff + nh * 1024:
                                       xoff + (nh + 1) * 1024],
                                in0=ps[:],
                                scalar1=0.0,
                            )
                        else:
                            nc.scalar.activation(
                                out=xt[:, xoff + nh * 1024:
                                       xoff + (nh + 1) * 1024],
                                in_=ps[:],
                                func=AF.Relu,
                            )

                # t3^T = (x2 @ W56)^T feature-major: lhsT = W56 DR chunks
                # (stationary [128,2,3]), rhs = x2 fp8 k-pairs (N=512)
                t3t_b = t3_pool.tile([3, V], bf16, tag="t3t")
                for h in range(2):
                    ps = psA.tile([3, 1024], f32, tag="psA")
                    for kc2 in range(2):
                        rhs3 = x2p[kc2][:].rearrange("p (j v) -> p j v", j=2)
                        for n2 in range(2):
                            col = h * 1024 + n2 * 512
                            nc.tensor.matmul(
                                out=ps[:, n2 * 512:(n2 + 1) * 512],
                                lhsT=w56_3d[:, kc2, :, :3],
                                rhs=rhs3[:, :, col:col + 512],
                                start=(kc2 == 0),
                                stop=(kc2 == 1),
                                perf_mode=DR,
                            )
                    nc.vector.tensor_copy(
                        out=t3t_b[:, h * 1024:(h + 1) * 1024], in_=ps[:]
                    )
                    nc.scalar.dma_start(
                        out=t3t_all[b * 3:(b + 1) * 3,
                                    h * 1024:(h + 1) * 1024],
                        in_=t3t_b[:, h * 1024:(h + 1) * 1024],
                    )

            # ---------- pair3 aggregation, all batches ----------
            # transpose t3^T -> vertex-major fp8 tiles [128, (b,cc)]
            t3_bf = const_pool.tile([P, NV2 * 32], fp8, tag="t3bf")
            for vc in range(NV):
                ps = psA.tile([P, G], bf16, tag="psA")
                nc.tensor.transpose(
                    out=ps[:],
                    in_=t3t_all[:, vc * P:(vc + 1) * P],
                    identity=ident_bf[:G, :G],
                )
                nc.vector.tensor_copy(
                    out=t3_bf[:, vc * 16:vc * 16 + G],
                    in_=ps[:],
                )
            # feature-major agg: z3^T[(b,cc), v] ; then relu
            x3t = const_pool.tile([G, V], bf16, tag="x3t")
            for h in range(2):
                ps = psA.tile([G, 1024], f32, tag="psA")
                t3_3d = t3_bf[:].rearrange("p (u j g) -> p u j g",
                                            u=NV2, j=2, g=16)
                for uc2 in range(NV2):
                    rhs3 = a2t[uc2][:].rearrange("p (j v) -> p j v", j=2)
                    for n2 in range(2):
                        col = h * 1024 + n2 * 512
                        nc.tensor.matmul(
                            out=ps[:, n2 * 512:(n2 + 1) * 512],
                            lhsT=t3_3d[:, uc2, :, :G],
                            rhs=rhs3[:, :, col:col + 512],
                            start=(uc2 == 0),
                            stop=(uc2 == NV2 - 1 and not has_bias3),
                            perf_mode=DR,
                        )
                if has_bias3:
                    for n2 in range(2):
                        col = h * 1024 + n2 * 512
                        nc.tensor.matmul(
                            out=ps[:, n2 * 512:(n2 + 1) * 512],
                            lhsT=biasp3[:],
                            rhs=rho1[:, col:col + 512],
                            start=False,
                            stop=True,
                        )
                nc.vector.tensor_scalar_max(
                    out=x3t[:, h * 1024:(h + 1) * 1024],
                    in0=ps[:],
                    scalar1=0.0,
                )
            # transpose back to vertex-major with (b,cc)->(cc,b) permute;
            # fp8: this is FC1's DoubleRow lhsT ([128, (kc2, j, b)] layout
            # == [128, (kc, b)] since the slot offset is kc*BL either way).
            x3_f8 = const_pool.tile([P, NV * 48], fp8, tag="x3f8")
            for dc in range(NV):
                ps = psA.tile([P, G], bf16, tag="psA")
                nc.tensor.transpose(
                    out=ps[:],
                    in_=x3t[:, dc * P:(dc + 1) * P],
                    identity=ident_bf[:G, :G],
                )
                nc.vector.tensor_copy(
                    out=x3_f8[:, dc * 48:(dc + 1) * 48]
                    .rearrange("p (c s) -> p c s", s=16)[:, :, :BL],
                    in_=ps[:].rearrange("p (b c) -> p c b", c=3),
                )

            # ---------- FC head (all batches together, fp8 DoubleRow) ----
            x3v = x3_f8[:].rearrange("p (k j s) -> p k j s", k=NK1DR, j=2)
            ps_h1 = psA.tile([BL, FC_H], f32, tag="psA")
            for kc2 in range(NK1DR):
                wt = stream_pool.tile([P, 2 * FC_H], fp8, tag="fcw")
                nc.sync.dma_start(out=wt[:], in_=d_fcw1[kc2])
                wv = wt[:].rearrange("p (j n) -> p j n", j=2)
                for n2 in range(2):
                    nc.tensor.matmul(
                        out=ps_h1[:, n2 * 512:(n2 + 1) * 512],
                        lhsT=x3v[:, kc2, :, :BL],
                        rhs=wv[:, :, n2 * 512:(n2 + 1) * 512],
                        start=(kc2 == 0),
                        stop=(kc2 == NK1DR - 1),
                        perf_mode=DR,
                    )
            h1 = hfin_pool.tile([BL, FC_H], bf16, tag="hfin")
            if has_fcb:
                fcb1_sb = hfin_pool.tile([BL, FC_H], f32, tag="fcb")
                nc.sync.dma_start(out=fcb1_sb[:], in_=d_fcb1[:])
                nc.vector.tensor_add(out=h1[:], in0=ps_h1[:], in1=fcb1_sb[:])
            else:
                nc.vector.tensor_copy(out=h1[:], in_=ps_h1[:])

            # transpose h1 -> h1T fp8 [128, (kc, b)] (FC2 DoubleRow lhsT);
            # bf16 transposes run single-pass (fp32 is two-pass LOW_HIGH).
            h1T = const_pool.tile([P, NKFC2 * 16], fp8, tag="h1T")
            for kc in range(NKFC2):
                ps = psA.tile([P, BL], bf16, tag="psA")
                nc.tensor.transpose(
                    out=ps[:],
                    in_=h1[:, kc * P:(kc + 1) * P],
                    identity=ident_bf[:BL, :BL],
                )
                nc.vector.tensor_copy(
                    out=h1T[:, kc * 16:kc * 16 + BL], in_=ps[:]
                )

            h1v = h1T[:].rearrange("p (k j s) -> p k j s", k=NK2DR, j=2)
            ps_h2 = psA.tile([BL, FC_H], f32, tag="psA")
            for kc2 in range(NK2DR):
                wt = stream_pool.tile([P, 2 * FC_H], fp8, tag="fcw")
                nc.sync.dma_start(out=wt[:], in_=d_fcw2[kc2])
                wv = wt[:].rearrange("p (j n) -> p j n", j=2)
                for n2 in range(2):
                    nc.tensor.matmul(
                        out=ps_h2[:, n2 * 512:(n2 + 1) * 512],
                        lhsT=h1v[:, kc2, :, :BL],
                        rhs=wv[:, :, n2 * 512:(n2 + 1) * 512],
                        start=(kc2 == 0),
                        stop=(kc2 == NK2DR - 1),
                        perf_mode=DR,
                    )
            h2 = hfin_pool.tile([BL, FC_H], bf16, tag="hfin")
            if has_fcb:
                fcb2_sb = hfin_pool.tile([BL, FC_H], f32, tag="fcb")
                nc.sync.dma_start(out=fcb2_sb[:], in_=d_fcb2[:])
                nc.vector.tensor_add(out=h2[:], in0=ps_h2[:], in1=fcb2_sb[:])
            else:
                nc.vector.tensor_copy(out=h2[:], in_=ps_h2[:])

            h2T = const_pool.tile([P, NKFC2 * 16], fp8, tag="h2T")
            for kc in range(NKFC2):
                ps = psA.tile([P, BL], bf16, tag="psA")
                nc.tensor.transpose(
                    out=ps[:],
                    in_=h2[:, kc * P:(kc + 1) * P],
                    identity=ident_bf[:BL, :BL],
                )
                nc.vector.tensor_copy(
                    out=h2T[:, kc * 16:kc * 16 + BL], in_=ps[:]
                )

            # FC3, software-pipelined: chunk ch's tanh/store tail is
            # emitted under chunk ch+1's matmuls.  The whole tail stays in
            # batch-major [BL, 1024] layout (OUT is [BL, FLAT]): no
            # transposes, and tanh reads the PSUM accumulator directly.
            nc.gpsimd.dma_start(out=vflat[:], in_=d_vflat[:])
            h2v = h2T[:].rearrange("p (k j s) -> p k j s", k=NK2DR, j=2)
            NCH = FLAT // FC_H  # 6
            ps_acc = [None] * NCH

            def fc3_tail(ch):
                cols = slice(ch * FC_H, (ch + 1) * FC_H)
                dch = hfin_pool.tile([BL, FC_H], f32, tag="dch",
                                     name=f"dch{ch}")
                if has_fcb:
                    fcb3_sb = hfin_pool.tile([BL, FC_H], f32, tag="fcb",
                                             name=f"fcb3_{ch}")
                    nc.gpsimd.dma_start(
                        out=fcb3_sb[:],
                        in_=d_fcb3[:, ch * FC_H:(ch + 1) * FC_H],
                    )
                    h3sb = hfin_pool.tile([BL, FC_H], f32, tag="hfin",
                                          name=f"h3sb{ch}")
                    nc.vector.tensor_add(
                        out=h3sb[:], in0=ps_acc[ch][:], in1=fcb3_sb[:]
                    )
                    nc.scalar.activation(out=dch[:], in_=h3sb[:],
                                         func=AF.Tanh)
                else:
                    nc.scalar.activation(out=dch[:], in_=ps_acc[ch][:],
                                         func=AF.Tanh)
                och = hfin_pool.tile([BL, FC_H], f32, tag="och",
                                     name=f"och{ch}")
                nc.vector.tensor_scalar_mul(
                    out=och[:], in0=dch[:], scalar1=0.1
                )
                nc.vector.tensor_add(
                    out=och[:], in0=och[:], in1=vflat[:, cols]
                )
                nc.sync.dma_start(out=d_out[:, cols], in_=och[:])

            for ch in range(NCH):
                ps = psA.tile([BL, FC_H], f32, tag="psA",
                              name=f"ps_fc3_{ch}")
                ps_acc[ch] = ps
                for kc2 in range(NK2DR):
                    wt = stream_pool.tile([P, 2 * FC_H], fp8, tag="fcw")
                    nc.sync.dma_start(out=wt[:], in_=d_fcw3[ch * NK2DR + kc2])
                    wv = wt[:].rearrange("p (j n) -> p j n", j=2)
                    for n2 in range(2):
                        nc.tensor.matmul(
                            out=ps[:, n2 * 512:(n2 + 1) * 512],
                            lhsT=h2v[:, kc2, :, :BL],
                            rhs=wv[:, :, n2 * 512:(n2 + 1) * 512],
                            start=(kc2 == 0),
                            stop=(kc2 == NK2DR - 1),
                            perf_mode=DR,
                        )
                if ch >= 1:
                    fc3_tail(ch - 1)
            fc3_tail(NCH - 1)

    nc.finalize()
    return nc


def build_in_maps(inputs):
    """Host prep + per-core input maps (exposed for testing)."""
    shared, per_core = _host_prep(inputs)
    key = (shared["HAS_BIAS1"], shared["HAS_BIAS2"], shared["HAS_BIAS3"],
           shared["HAS_FCB"])
    shared_arrays = {k: v for k, v in shared.items() if isinstance(v, np.ndarray)}
    in_maps = []
    for c in range(N_CORES):
        m = dict(shared_arrays)
        m.update(per_core[c])
        in_maps.append(m)
    return key, in_maps


def unpack_out(raw):
    return np.asarray(raw, np.float32).reshape(BL, V, 3)


def kernel(**inputs):
    key, in_maps = build_in_maps(inputs)
    if key not in _CACHE:
        _CACHE[key] = _build_program(*key)
    nc = _CACHE[key]

    from concourse.bass_utils import run_bass_kernel_spmd

    res = run_bass_kernel_spmd(nc, in_maps, list(range(N_CORES)))
    out = np.empty((B, V, 3), np.float32)
    for c in range(N_CORES):
        out[c * BL:(c + 1) * BL] = unpack_out(res.results[c]["OUT"])
    return out


# revision 27
# speedup vs baseline: 1.0325x; 1.0130x over previous
"""Trainium2 Bass kernel for nn_GCNModel (6-layer GCN + 3-layer FC mesh deformer).

Strategy
--------
Data-parallel over batch B=32 across 8 NeuronCores (4 batch elements each).

Algebraic restructuring (host side, exact):
  ReLU only follows GCN layers 2, 4, 6, so each pair of GCN layers collapses:
      A(A x W1 + 1 b1^T) W2 + 1 b2^T
        = A^2 x (W1 W2) + (A 1) (b1 W2)^T + 1 b2^T
  with A the dense-ified normalized adjacency.  Three aggregations with a
  host-precomputed dense A^2 replace six sparse gather/scatter aggregations.
  Further:
    * pair 1's aggregation input is rank-3 (x = [verts | 1 img^T]):
      A^2 x W12 = (A^2 verts) W12[:3] + (A^2 1) (img W12[3:])^T
      so the wide aggregation reduces to a width-3 one plus rank-1 terms,
      all folded into ONE k=4 (k=6 with biases) matmul per output tile.
    * pair 3 aggregates after the [512,3] transform (width 3).
  Only pair 2 needs a full width-512 dense A^2 apply per batch element.

Everything on the critical path runs in fp8 (e4m3) DoubleRow matmuls with
fp32 PSUM accumulation where the layout permits: the A^2 aggregations, the
W34 transform, the W56 transform, and the whole FC head (weights, x3, h1,
h2 all fp8).  Host-validated vs the fp32 reference: ~6e-3 max relative
error (the output is dominated by `vertices` plus a 0.1-scaled
tanh-squashed deformation).

Layouts alternate vertex-major / feature-major so no transposes are needed
in the hot path:
  agg (contracts over vertices):  lhsT = t (vertex-major), rhs = A2T rows
                                  -> feature-major output
  transform (contracts over features): lhsT = x (feature-major), rhs = W
                                  -> vertex-major output
"""

import numpy as np
import ml_dtypes

B, V, E, IMG_F = 32, 2048, 12288, 512
N_CORES = 8
BL = B // N_CORES  # 4 batch elements per core
P = 128
NV = V // P   # 16 vertex chunks
F = 512
NF = F // P   # 4 feature chunks
FC_H = 1024
FLAT = V * 3  # 6144
NKFC1 = FLAT // P  # 48
NKFC2 = FC_H // P  # 8
NV2 = NV // 2  # 8 double-row vertex chunks
NK1DR = NKFC1 // 2  # 24 DoubleRow k-tiles for FC1
NK2DR = NKFC2 // 2  # 4 DoubleRow k-tiles for FC2/FC3

BF16 = ml_dtypes.bfloat16
FP8 = ml_dtypes.float8_e4m3

_CACHE = {}


def _host_prep(inputs):
    """Exact (fp64) host-side algebra: dense A^2, collapsed weights, shards."""
    ei = np.asarray(inputs["edge_index"])
    src = np.concatenate([ei[0], np.arange(V)]).astype(np.int64)
    dst = np.concatenate([ei[1], np.arange(V)]).astype(np.int64)
    deg = np.zeros(V)
    np.add.at(deg, dst, 1.0)
    dinv = 1.0 / np.sqrt(deg)
    normv = dinv[src] * dinv[dst]
    A = np.zeros((V, V))
    np.add.at(A, (dst, src), normv)
    A2 = A @ A
    rho = (A @ np.ones(V)).astype(np.float32)
    rho2 = (A2 @ np.ones(V)).astype(np.float32)

    W = [np.asarray(inputs[f"W{i}"], np.float64) for i in range(1, 7)]
    bb = [np.asarray(inputs[f"b{i}"], np.float64) for i in range(1, 7)]
    W12 = W[0] @ W[1]
    W34 = W[2] @ W[3]
    W56 = W[4] @ W[5]
    bias1 = bb[0] @ W[1]  # pairs with rho
    bias2 = bb[2] @ W[3]
    bias3 = bb[4] @ W[5]
    b2, b4, b6 = bb[1], bb[3], bb[5]

    shared = {}
    # A2T in fp8 DoubleRow layout: [uc2][p, j*V + v] = A2T[uc2*256+j*128+p, v]
    A2T = np.ascontiguousarray(A2.T).astype(np.float32)
    shared["A2T"] = np.ascontiguousarray(
        A2T.reshape(NV2, 2, P, V).transpose(0, 2, 1, 3).reshape(NV2, P, 2 * V)
    ).astype(FP8)
    # k=3 static lhsT rows for pair1 (verts); c1 (img term) is folded in as
    # lhsT row 3 (rhs row 3 = rho2), biases as rows 4-5 (rhs rows = rho1).
    shared["W12A"] = np.asarray(W12[:3], np.float32).astype(BF16)
    bias_pack1 = np.stack([bias1, b2]).astype(np.float32)  # pairs with rho1
    shared["HAS_BIAS1"] = bool(np.any(bias_pack1))
    shared["BIASP1"] = bias_pack1.astype(BF16)
    shared["RHO2"] = rho2.reshape(1, V).astype(BF16)
    shared["RHO1"] = np.stack([rho, np.ones(V, np.float32)]).astype(BF16)

    def pack_rows(w, ncol):
        # [nk*128, ncol] -> [128, nk*ncol] with chunk kc at cols [kc*ncol:...]
        w = np.asarray(w, np.float32)
        nk = w.shape[0] // P
        return np.ascontiguousarray(
            w.reshape(nk, P, ncol).transpose(1, 0, 2).reshape(P, nk * ncol)
        )

    def pack_dr(w, ncol):
        # [nk2*256, ncol] -> [nk2, 128, 2*ncol]: tile i, row p, col j*ncol+n
        # = w[i*256 + j*128 + p, n]  (DoubleRow k-pair layout)
        w = np.asarray(w, np.float32)
        nk2 = w.shape[0] // 256
        return np.ascontiguousarray(
            w.reshape(nk2, 2, P, ncol).transpose(0, 2, 1, 3)
            .reshape(nk2, P, 2 * ncol)
        )

    shared["W12B"] = pack_rows(W12[3:], F).astype(BF16)
    # W34 in fp8 DoubleRow layout: [p, (fc2, j, fout)] = W34[fc2*256+j*128+p, f]
    W34f = np.asarray(W34, np.float32)
    shared["W34"] = np.ascontiguousarray(
        W34f.reshape(2, 2, P, F).transpose(2, 0, 1, 3).reshape(P, 4 * F)
    ).astype(FP8)
    # W56 fp8 DoubleRow: [p, (kc2, j, c-slot16)] = W56[kc2*256+j*128+p, c]
    # (c slot padded 3->16: dual-fp8 LDW requires 16B-aligned j-stride)
    w56r = np.zeros((2, 2, P, 16), np.float32)
    w56r[:, :, :, :3] = np.asarray(W56, np.float32).reshape(2, 2, P, 3)
    shared["W56"] = np.ascontiguousarray(
        w56r.transpose(2, 0, 1, 3).reshape(P, 64)
    ).astype(FP8)

    # pair2/3 bias packs (zero in the shipped model; matmul-folded if not)
    bias_pack2 = np.stack([bias2, b4]).astype(np.float32)  # [2, 512]
    bias_pack3 = np.zeros((2, BL * 3), np.float32)
    for b in range(BL):
        bias_pack3[0, b * 3:(b + 1) * 3] = bias3
        bias_pack3[1, b * 3:(b + 1) * 3] = b6
    shared["HAS_BIAS2"] = bool(np.any(bias_pack2))
    shared["HAS_BIAS3"] = bool(np.any(bias_pack3))
    shared["BIASP2"] = bias_pack2.astype(BF16)
    shared["BIASP3"] = bias_pack3.astype(BF16)

    # FC weights, fp8 DoubleRow tiles. fcW1 rows permuted: new row
    # (vc*3+c)*128+p corresponds to original row (vc*128+p)*3+c.
    fcW1 = np.asarray(inputs["fcW1"], np.float32)
    idx = (
        (np.arange(NV)[:, None, None] * P + np.arange(P)[None, None, :]) * 3
        + np.arange(3)[None, :, None]
    ).reshape(-1)  # (vc, c, p) -> orig row
    shared["FCW1"] = pack_dr(fcW1[idx], FC_H).astype(FP8)  # [24, 128, 2048]
    shared["FCW2"] = pack_dr(
        np.asarray(inputs["fcW2"], np.float32), FC_H
    ).astype(FP8)  # [4, 128, 2048]
    # FCW3: [(ch*4+kc2), p, j*1024+n] = fcW3[kc2*256+j*128+p, ch*1024+n]
    fcW3 = np.asarray(inputs["fcW3"], np.float32)
    shared["FCW3"] = np.ascontiguousarray(
        fcW3.reshape(NK2DR, 2, P, FLAT // FC_H, FC_H)
        .transpose(3, 0, 2, 1, 4).reshape(24, P, 2 * FC_H)
    ).astype(FP8)
    fcb1 = np.asarray(inputs["fcb1"], np.float32)
    fcb2 = np.asarray(inputs["fcb2"], np.float32)
    fcb3 = np.asarray(inputs["fcb3"], np.float32)
    shared["HAS_FCB"] = bool(np.any(fcb1) or np.any(fcb2) or np.any(fcb3))
    shared["FCB1"] = np.ascontiguousarray(np.broadcast_to(fcb1, (BL, FC_H)))
    shared["FCB2"] = np.ascontiguousarray(np.broadcast_to(fcb2, (BL, FC_H)))
    shared["FCB3"] = np.ascontiguousarray(np.broadcast_to(fcb3, (BL, FLAT)))

    # W12A16DR (no-bias x1 lhsT, fp8 DoubleRow over the 16-row phase0
    # output): [b][p, j*F+f] = w12a16[b][j*8+p, f], with W12A rows at
    # g=b*4+c, zeros elsewhere; the c1 slot g=b*4+3 is filled on device.
    # The j*8+p pairing lets the avt repack DMA read plain partition
    # ranges (avt rows 0-7 -> j=0, rows 8-15 -> j=1).
    w12a16 = np.zeros((BL, 16, F), np.float32)
    for b in range(BL):
        w12a16[b, b * 4:b * 4 + 3, :] = np.asarray(W12[:3], np.float32)
    shared["W12A16DR"] = np.ascontiguousarray(
        w12a16.reshape(BL, 2, 8, F).transpose(0, 2, 1, 3)
        .reshape(BL, 8, 2 * F)
    ).astype(FP8)

    # per-core shards
    verts = np.asarray(inputs["vertices"], np.float32)  # [B, V, 3]
    img = np.asarray(inputs["img_features"], np.float32)  # [B, 512]
    per_core = []
    for c in range(N_CORES):
        vb = verts[c * BL:(c + 1) * BL]  # [BL, V, 3]
        # DoubleRow lhsT: [uc2][p, j*16 + (b*4+cc)] = verts[b, uc2*256+j*128+p, cc]
        # with a ones column at g=b*4+3 so phase0 emits rho2 = A^2 @ 1 as
        # row 3 of every batch group.  (16B-aligned j-stride for dual-fp8.)
        vraw = vb.transpose(1, 0, 2).reshape(NV2, 2, P, BL, 3)
        vvm = np.zeros((NV2, P, 2, BL, 4), np.float32)
        vvm[:, :, :, :, :3] = vraw.transpose(0, 2, 1, 3, 4)
        vvm[:, :, :, :, 3] = 1.0
        vvm = np.ascontiguousarray(vvm.reshape(NV2, P, 32)).astype(FP8)
        per_core.append({
            "VVM": vvm,
            "VFLAT": np.ascontiguousarray(vb.reshape(BL, FLAT)),
            "IMG": np.ascontiguousarray(img[c * BL:(c + 1) * BL]).astype(BF16),
        })
    return shared, per_core


def _build_program(has_bias1, has_bias2, has_bias3, has_fcb):
    """Emit the Bass/Tile program (identical on all cores)."""
    from concourse import bacc, bass, mybir, tile
    from concourse.masks import make_identity

    f32 = mybir.dt.float32
    bf16 = mybir.dt.bfloat16
    fp8 = mybir.dt.float8e4
    AF = mybir.ActivationFunctionType
    DR = mybir.MatmulPerfMode.DoubleRow

    nc = bacc.Bacc(trn_type="TRN2")

    d_a2t = nc.dram_tensor("A2T", [NV2, P, 2 * V], fp8, kind="ExternalInput")
    d_w12a = nc.dram_tensor("W12A", [3, F], bf16, kind="ExternalInput")
    d_w12a16 = nc.dram_tensor("W12A16DR", [BL, 8, 2 * F], fp8, kind="ExternalInput")
    d_biasp1 = nc.dram_tensor("BIASP1", [2, F], bf16, kind="ExternalInput")
    d_rho2 = nc.dram_tensor("RHO2", [1, V], bf16, kind="ExternalInput")
    d_rho1 = nc.dram_tensor("RHO1", [2, V], bf16, kind="ExternalInput")
    d_w12b = nc.dram_tensor("W12B", [P, 4 * F], bf16, kind="ExternalInput")
    d_w34 = nc.dram_tensor("W34", [P, 4 * F], fp8, kind="ExternalInput")
    d_w56 = nc.dram_tensor("W56", [P, 64], fp8, kind="ExternalInput")
    d_biasp2 = nc.dram_tensor("BIASP2", [2, F], bf16, kind="ExternalInput")
    d_biasp3 = nc.dram_tensor("BIASP3", [2, BL * 3], bf16, kind="ExternalInput")
    d_fcw1 = nc.dram_tensor("FCW1", [NK1DR, P, 2 * FC_H], fp8, kind="ExternalInput")
    d_fcw2 = nc.dram_tensor("FCW2", [NK2DR, P, 2 * FC_H], fp8, kind="ExternalInput")
    d_fcw3 = nc.dram_tensor("FCW3", [24, P, 2 * FC_H], fp8, kind="ExternalInput")
    d_fcb1 = nc.dram_tensor("FCB1", [BL, FC_H], f32, kind="ExternalInput")
    d_fcb2 = nc.dram_tensor("FCB2", [BL, FC_H], f32, kind="ExternalInput")
    d_fcb3 = nc.dram_tensor("FCB3", [BL, FLAT], f32, kind="ExternalInput")
    d_vvm = nc.dram_tensor("VVM", [NV2, P, 32], fp8, kind="ExternalInput")
    d_vflat = nc.dram_tensor("VFLAT", [BL, FLAT], f32, kind="ExternalInput")
    d_img = nc.dram_tensor("IMG", [BL, IMG_F], bf16, kind="ExternalInput")
    d_out = nc.dram_tensor("OUT", [BL, FLAT], f32, kind="ExternalOutput")

    G = BL * 3  # 12: per-vertex-chunk group width (batch x coord)
    KX1 = 6 if has_bias1 else 4  # x1 folded-matmul contraction depth

    with tile.TileContext(nc) as tc:
        with (
            tc.tile_pool(name="const", bufs=1) as const_pool,
            tc.tile_pool(name="x", bufs=2) as x_pool,
            tc.tile_pool(name="tbf", bufs=1) as tbf_pool,
            tc.tile_pool(name="work", bufs=1) as work_pool,
            tc.tile_pool(name="t3p", bufs=2) as t3_pool,
            tc.tile_pool(name="stream", bufs=32) as stream_pool,
            tc.tile_pool(name="hfin", bufs=2) as hfin_pool,
            tc.tile_pool(name="psA", bufs=3, space="PSUM") as psA,
            tc.tile_pool(name="psB", bufs=2, space="PSUM") as psB,
        ):
            # ---------- resident constants ----------
            # Everything DMA-critical rides the sync queue in need-order:
            # ~0.9MB of small constants (2.5us), then the 4.2MB A2T stream
            # that paces phase0, then the FC weight streams.  Serializing
            # them on one queue is deliberate: parallel queues would split
            # HBM bandwidth and delay A2T, which gates batch 0's z2.
            vvm = const_pool.tile([P, NV2 * 32], fp8, tag="vvm")
            for uc2 in range(NV2):
                nc.sync.dma_start(
                    out=vvm[:, uc2 * 32:(uc2 + 1) * 32], in_=d_vvm[uc2]
                )
            a2t = []
            for uc2 in range(NV2):
                t = const_pool.tile([P, 2 * V], fp8, tag=f"a2t{uc2}")
                nc.sync.dma_start(out=t[:], in_=d_a2t[uc2])
                a2t.append(t)
            # w12b/w34 ride behind A2T: c1 needs w12b only at ~21us
            # (between the phase0 halves) and t2 needs w34 at ~33us.
            w12b = const_pool.tile([P, 4 * F], bf16, tag="w12b")
            nc.sync.dma_start(out=w12b[:], in_=d_w12b[:])
            w34 = const_pool.tile([P, 4 * F], fp8, tag="w34")
            nc.sync.dma_start(out=w34[:], in_=d_w34[:])

            # identities first: make_identity runs on the gpsimd engine and
            # must precede the gpsimd DMA triggers below, which would
            # otherwise delay the PE warm-up transpose by ~10us.
            ident = const_pool.tile([P, P], f32, tag="ident")
            make_identity(nc, ident[:])
            ident_bf = const_pool.tile([P, P], bf16, tag="ident_bf")
            make_identity(nc, ident_bf[:])

            # remaining small operands on the gpsimd queue (idle
            # otherwise; all fire within the first ~6us).

            # dummy transpose: absorbs the gpsimd(identity) wait on the PE
            # clock -- walrus allows only ONE sync wait on transpose-mode
            # matmuls (S3 LW struct), so later transposes must carry only
            # their data dependency.
            ps_warm = psA.tile([1, P], f32, tag="psA")
            nc.tensor.transpose(
                out=ps_warm[:], in_=ident[:, 0:1], identity=ident[:]
            )
            vflat = const_pool.tile([BL, FLAT], f32, tag="vflat")
            nc.vector.tensor_copy(out=vflat[0:1, 0:P], in_=ps_warm[:])
            # HAM warm-up: dummy matmuls on the identity while the A2T
            # tiles stream in; keeps the PE activity monitor at K=8/8 so
            # the real aggregation starts at 2.4 GHz instead of 1.2.
            ps_w2 = psB.tile([P, F], f32, tag="psB")

            def ham_keepalive(n):
                for _ in range(n):
                    nc.tensor.matmul(
                        out=ps_w2[:, :P],
                        lhsT=ident_bf[:],
                        rhs=ident_bf[:],
                        start=True,
                        stop=True,
                    )

            ham_keepalive(40)
            nc.vector.tensor_copy(out=vflat[0:1, 0:P], in_=ps_w2[:1, :P])

            # x1 operands.  No-bias path: phase0 emits [16, V] batch
            # groups [av_b(3); rho2] directly (ones column in vvm), and x1
            # contracts over all 16 rows with a per-batch lhsT whose other
            # batches' rows are zero -- no post-phase0 gather DMAs at all.
            # Bias path (unused in the shipped model): per-batch [6, V]
            # rhs assembly as before.
            img_all = const_pool.tile([P, NF * BL], bf16, tag="img_all")
            for b in range(BL):
                nc.gpsimd.dma_start(
                    out=img_all[:].rearrange("p (k b) -> p k b", k=NF)[:, :, b],
                    in_=d_img[b].rearrange("(k p) -> p k", p=P),
                )
            av4 = []
            lhsT4 = []
            lhsT16 = []
            for b in range(BL):
                if has_bias1:
                    avb = const_pool.tile([KX1, V], bf16, tag=f"av4_{b}")
                    nc.gpsimd.dma_start(out=avb[3:4, :], in_=d_rho2[:])
                    nc.gpsimd.dma_start(out=avb[4:6, :], in_=d_rho1[:])
                    av4.append(avb)
                    lb = const_pool.tile([KX1, F], bf16, tag=f"lhsT4_{b}")
                    nc.gpsimd.dma_start(out=lb[0:3, :], in_=d_w12a[:])
                    nc.gpsimd.dma_start(out=lb[4:6, :], in_=d_biasp1[:])
                    lhsT4.append(lb)
                else:
                    lb = const_pool.tile([8, 2 * F], fp8, tag=f"lhsT16_{b}")
                    nc.gpsimd.dma_start(out=lb[:], in_=d_w12a16[b])
                    lhsT16.append(lb)
            w56 = const_pool.tile([P, 64], fp8, tag="w56")
            nc.gpsimd.dma_start(out=w56[:], in_=d_w56[:])
            if has_bias2 or has_bias3:
                rho1 = const_pool.tile([2, V], bf16, tag="rho1")
                nc.gpsimd.dma_start(out=rho1[:], in_=d_rho1[:])
            if has_bias2:
                biasp2 = const_pool.tile([2, F], bf16, tag="biasp2")
                nc.gpsimd.dma_start(out=biasp2[:], in_=d_biasp2[:])
            if has_bias3:
                biasp3 = const_pool.tile([2, BL * 3], bf16, tag="biasp3")
                nc.gpsimd.dma_start(out=biasp3[:], in_=d_biasp3[:])

            def emit_c1():
                # c1[b] = img_b @ W12b -> [BL, 512], emitted between the
                # two phase0 halves (needs only img+w12b, which land ~8us;
                # must not gate phase0's DMA-paced h=0 pass).
                img3 = img_all[:].rearrange("p (k b) -> p k b", k=NF)
                ps_c1 = psB.tile([BL, F], f32, tag="psB")
                for kc in range(NF):
                    nc.tensor.matmul(
                        out=ps_c1[:],
                        lhsT=img3[:, kc],
                        rhs=w12b[:, kc * F:(kc + 1) * F],
                        start=(kc == 0),
                        stop=(kc == NF - 1),
                    )
                c1_all = work_pool.tile(
                    [BL, F], bf16 if has_bias1 else fp8, tag="c1"
                )
                nc.vector.tensor_copy(out=c1_all[:], in_=ps_c1[:])
                for b in range(BL):
                    # no-bias: c1 slot is g=b*4+3 -> p=g%8, j=g//8
                    g = 4 * b + 3
                    dst = (lhsT4[b][3:4, :] if has_bias1
                           else lhsT16[b][g % 8:g % 8 + 1,
                                          (g // 8) * F:(g // 8 + 1) * F])
                    nc.scalar.dma_start(out=dst, in_=c1_all[b:b + 1, :])

            # ---------- phase 0: verts aggregation, feature-major ----------
            # av^T[(b,cc), v] = sum_u verts[u,(b,cc)] * A2T[u, v] : lhsT = vvm
            # chunks (stationary, tiny), rhs = A2T rows (N=512 streams).
            # Wide-N streaming; also lets PE start as soon as a2t[0] lands.
            avt_bf = const_pool.tile([16, V], bf16 if has_bias1 else fp8,
                                     tag="avt")
            if not has_bias1:
                # x1's DoubleRow rhs: row g=j*8+p at partition p, half j.
                avt_dr = const_pool.tile([8, 2 * V], fp8, tag="avtdr")
                avt_dr3 = avt_dr[:].rearrange("p (j v) -> p j v", j=2)
            for h in range(2):
                ps = psA.tile([16, 1024], f32, tag="psA")
                for uc2 in range(NV2):
                    lhsT = vvm[:, uc2 * 32:(uc2 + 1) * 32].rearrange(
                        "p (j g) -> p j g", j=2
                    )
                    rhs3 = a2t[uc2][:].rearrange("p (j v) -> p j v", j=2)
                    for n2 in range(2):
                        col = h * 1024 + n2 * 512
                        nc.tensor.matmul(
                            out=ps[:, n2 * 512:(n2 + 1) * 512],
                            lhsT=lhsT,
                            rhs=rhs3[:, :, col:col + 512],
                            start=(uc2 == 0),
                            stop=(uc2 == NV2 - 1),
                            perf_mode=DR,
                        )
                nc.vector.tensor_copy(
                    out=avt_bf[:, h * 1024:(h + 1) * 1024], in_=ps[:]
                )
                if not has_bias1:
                    for j in range(2):
                        nc.scalar.dma_start(
                            out=avt_dr3[:, j, h * 1024:(h + 1) * 1024],
                            in_=avt_bf[j * 8:j * 8 + 8,
                                       h * 1024:(h + 1) * 1024],
                        )
                if h == 0:
                    emit_c1()
                if has_bias1:
                    # per-batch row triples -> rows 0-2 of the per-batch
                    # rhs tiles (SBUF->SBUF DMA: compute engines cannot
                    # address partition offsets not in {0,32,64,96})
                    for b in range(BL):
                        nc.scalar.dma_start(
                            out=av4[b][0:3, h * 1024:(h + 1) * 1024],
                            in_=avt_bf[b * 4:b * 4 + 3,
                                       h * 1024:(h + 1) * 1024],
                        )

            # t3 storage across batches, feature-major [(b,cc), v] f32
            t3t_all = const_pool.tile([G, V], bf16, tag="t3t_all")

            w56_3d = w56[:].rearrange("p (k j c) -> p k j c", k=2, j=2)  # c slot = 16
            w34_3d = w34[:].rearrange("p (k j n) -> p k j n", k=2, j=2)

            # ---------- per batch: pair1 -> pair2 -> t3 ----------
            for b in range(BL):
                # x1 feature-major [f, v] = relu(single k=4/6 matmul folding
                #   verts agg + image rank-1 term [+ biases]), fp8
                x1_all = tbf_pool.tile([P, NF * V], fp8, tag="x1")
                x1_3d = x1_all[:].rearrange("p (f v) -> p f v", f=NF)
                for fc in range(NF):
                    for nh in range(2):
                        ps = psA.tile([P, 1024], f32, tag="psA")
                        col = nh * 1024
                        if has_bias1:
                            for n2 in range(2):
                                nc.tensor.matmul(
                                    out=ps[:, n2 * 512:(n2 + 1) * 512],
                                    lhsT=lhsT4[b][:, fc * P:(fc + 1) * P],
                                    rhs=av4[b][:, col + n2 * 512:
                                               col + (n2 + 1) * 512],
                                    start=True,
                                    stop=True,
                                )
                        else:
                            lhsT_x1 = lhsT16[b][:].rearrange(
                                "p (j f) -> p j f", j=2
                            )[:, :, fc * P:(fc + 1) * P]
                            for n2 in range(2):
                                nc.tensor.matmul(
                                    out=ps[:, n2 * 512:(n2 + 1) * 512],
                                    lhsT=lhsT_x1,
                                    rhs=avt_dr3[:, :, col + n2 * 512:
                                                col + (n2 + 1) * 512],
                                    start=True,
                                    stop=True,
                                    perf_mode=DR,
                                )
                        if nh == 0:
                            nc.vector.tensor_scalar_max(
                                out=x1_all[:, fc * V + nh * 1024:
                                           fc * V + (nh + 1) * 1024],
                                in0=ps[:],
                                scalar1=0.0,
                            )
                        else:
                            nc.scalar.activation(
                                out=x1_all[:, fc * V + nh * 1024:
                                           fc * V + (nh + 1) * 1024],
                                in_=ps[:],
                                func=AF.Relu,
                            )

                # t2 vertex-major fp8 [v, f] via DoubleRow over k=f
                t2_f8 = tbf_pool.tile([P, NV * F], fp8, tag="t2")
                for vc in range(NV):
                    ps = psB.tile([P, F], f32, tag="psB")
                    for fc2 in range(2):
                        nc.tensor.matmul(
                            out=ps[:],
                            lhsT=x1_3d[:, fc2 * 2:fc2 * 2 + 2,
                                       vc * P:(vc + 1) * P],
                            rhs=w34_3d[:, fc2],
                            start=(fc2 == 0),
                            stop=(fc2 == 1),
                            perf_mode=DR,
                        )
                    if vc % 2 == 0:
                        nc.vector.tensor_copy(
                            out=t2_f8[:, vc * F:(vc + 1) * F], in_=ps[:]
                        )
                    else:
                        nc.scalar.copy(
                            out=t2_f8[:, vc * F:(vc + 1) * F], in_=ps[:]
                        )

                # z2 = A^2 t2 (feature-major out, DoubleRow) ; x2 = relu,
                # stored fp8 as DoubleRow k-pairs for the t3 transform.
                t2_3d = t2_f8[:].rearrange("p (u j f) -> p u j f", u=NV2, j=2)
                x2p = [x_pool.tile([P, 2 * V], fp8, tag="x", name=f"x2p{i}")
                       for i in range(2)]
                for fc in range(NF):
                    xoff = (fc % 2) * V
                    xt = x2p[fc // 2]
                    for nh in range(2):
                        ps = psA.tile([P, 1024], f32, tag="psA")
                        for uc2 in range(NV2):
                            lhsT = t2_3d[:, uc2, :, fc * P:(fc + 1) * P]
                            rhs3 = a2t[uc2][:].rearrange(
                                "p (j v) -> p j v", j=2
                            )
                            for n2 in range(2):
                                col = nh * 1024 + n2 * 512
                                nc.tensor.matmul(
                                    out=ps[:, n2 * 512:(n2 + 1) * 512],
                                    lhsT=lhsT,
                                    rhs=rhs3[:, :, col:col + 512],
                                    start=(uc2 == 0),
                                    stop=(uc2 == NV2 - 1 and not has_bias2),
                                    perf_mode=DR,
                                )
                        if has_bias2:
                            for n2 in range(2):
                                col = nh * 1024 + n2 * 512
                                nc.tensor.matmul(
                                    out=ps[:, n2 * 512:(n2 + 1) * 512],
                                    lhsT=biasp2[:, fc * P:(fc + 1) * P],
                                    rhs=rho1[:, col:col + 512],
                                    start=False,
                                    stop=True,
                                )
                        if nh == 0:
                            nc.vector.tensor_scalar_max(
                                out=xt[:, xoff + nh * 1024:
                                       xoff + (nh + 1) * 1024],
                                in0=ps[:],
                                scalar1=0.0,
                            )
                        else:
                            nc.scalar.activation(
                                out=xt[:, xoff + nh * 1024:
                                       xoff + (nh + 1) * 1024],
                                in_=ps[:],
                                func=AF.Relu,
                            )

                # t3^T = (x2 @ W56)^T feature-major: lhsT = W56 DR chunks
                # (stationary [128,2,3]), rhs = x2 fp8 k-pairs (N=512)
                t3t_b = t3_pool.tile([3, V], bf16, tag="t3t")
                for h in range(2):
                    ps = psA.tile([3, 1024], f32, tag="psA")
                    for kc2 in range(2):
                        rhs3 = x2p[kc2][:].rearrange("p (j v) -> p j v", j=2)
                        for n2 in range(2):
                            col = h * 1024 + n2 * 512
                            nc.tensor.matmul(
                                out=ps[:, n2 * 512:(n2 + 1) * 512],
                                lhsT=w56_3d[:, kc2, :, :3],
                                rhs=rhs3[:, :, col:col + 512],
                                start=(kc2 == 0),
                                stop=(kc2 == 1),
                                perf_mode=DR,
                            )
                    nc.vector.tensor_copy(
                        out=t3t_b[:, h * 1024:(h + 1) * 1024], in_=ps[:]
                    )
                    nc.scalar.dma_start(
                        out=t3t_all[b * 3:(b + 1) * 3,
                                    h * 1024:(h + 1) * 1024],
                        in_=t3t_b[:, h * 1024:(h + 1) * 1024],
                    )

            # ---------- pair3 aggregation, all batches ----------
            # transpose t3^T -> vertex-major fp8 tiles [128, (b,cc)]
            t3_bf = const_pool.tile([P, NV2 * 32], fp8, tag="t3bf")
            for vc in range(NV):
                ps = psA.tile([P, G], bf16, tag="psA")
                nc.tensor.transpose(
                    out=ps[:],
                    in_=t3t_all[:, vc * P:(vc + 1) * P],
                    identity=ident_bf[:G, :G],
                )
                nc.vector.tensor_copy(
                    out=t3_bf[:, vc * 16:vc * 16 + G],
                    in_=ps[:],
                )
            # feature-major agg: z3^T[(b,cc), v] ; then relu
            x3t = const_pool.tile([G, V], bf16, tag="x3t")
            for h in range(2):
                ps = psA.tile([G, 1024], f32, tag="psA")
                t3_3d = t3_bf[:].rearrange("p (u j g) -> p u j g",
                                            u=NV2, j=2, g=16)
                for uc2 in range(NV2):
                    rhs3 = a2t[uc2][:].rearrange("p (j v) -> p j v", j=2)
                    for n2 in range(2):
                        col = h * 1024 + n2 * 512
                        nc.tensor.matmul(
                            out=ps[:, n2 * 512:(n2 + 1) * 512],
                            lhsT=t3_3d[:, uc2, :, :G],
                            rhs=rhs3[:, :, col:col + 512],
                            start=(uc2 == 0),
                            stop=(uc2 == NV2 - 1 and not has_bias3),
                            perf_mode=DR,
                        )
                if has_bias3:
                    for n2 in range(2):
                        col = h * 1024 + n2 * 512
                        nc.tensor.matmul(
                            out=ps[:, n2 * 512:(n2 + 1) * 512],
                            lhsT=biasp3[:],
                            rhs=rho1[:, col:col + 512],
                            start=False,
                            stop=True,
                        )
                nc.vector.tensor_scalar_max(
                    out=x3t[:, h * 1024:(h + 1) * 1024],
                    in0=ps[:],
                    scalar1=0.0,
                )
            # transpose back to vertex-major with (b,cc)->(cc,b) permute;
            # fp8: this is FC1's DoubleRow lhsT ([128, (kc2, j, b)] layout
            # == [128, (kc, b)] since the slot offset is kc*BL either way).
            x3_f8 = const_pool.tile([P, NV * 48], fp8, tag="x3f8")
            for dc in range(NV):
                ps = psA.tile([P, G], bf16, tag="psA")
                nc.tensor.transpose(
                    out=ps[:],
                    in_=x3t[:, dc * P:(dc + 1) * P],
                    identity=ident_bf[:G, :G],
                )
                nc.vector.tensor_copy(
                    out=x3_f8[:, dc * 48:(dc + 1) * 48]
                    .rearrange("p (c s) -> p c s", s=16)[:, :, :BL],
                    in_=ps[:].rearrange("p (b c) -> p c b", c=3),
                )

            # ---------- FC head (all batches together, fp8 DoubleRow) ----
            x3v = x3_f8[:].rearrange("p (k j s) -> p k j s", k=NK1DR, j=2)
            ps_h1 = psA.tile([BL, FC_H], f32, tag="psA")
            for kc2 in range(NK1DR):
                wt = stream_pool.tile([P, 2 * FC_H], fp8, tag="fcw")
                nc.sync.dma_start(out=wt[:], in_=d_fcw1[kc2])
                wv = wt[:].rearrange("p (j n) -> p j n", j=2)
                for n2 in range(2):
                    nc.tensor.matmul(
                        out=ps_h1[:, n2 * 512:(n2 + 1) * 512],
                        lhsT=x3v[:, kc2, :, :BL],
                        rhs=wv[:, :, n2 * 512:(n2 + 1) * 512],
                        start=(kc2 == 0),
                        stop=(kc2 == NK1DR - 1),
                        perf_mode=DR,
                    )
            h1 = hfin_pool.tile([BL, FC_H], bf16, tag="hfin")
            if has_fcb:
                fcb1_sb = hfin_pool.tile([BL, FC_H], f32, tag="fcb")
                nc.sync.dma_start(out=fcb1_sb[:], in_=d_fcb1[:])
                nc.vector.tensor_add(out=h1[:], in0=ps_h1[:], in1=fcb1_sb[:])
            else:
                nc.vector.tensor_copy(out=h1[:], in_=ps_h1[:])

            # transpose h1 -> h1T fp8 [128, (kc, b)] (FC2 DoubleRow lhsT);
            # bf16 transposes run single-pass (fp32 is two-pass LOW_HIGH).
            h1T = const_pool.tile([P, NKFC2 * 16], fp8, tag="h1T")
            for kc in range(NKFC2):
                ps = psA.tile([P, BL], bf16, tag="psA")
                nc.tensor.transpose(
                    out=ps[:],
                    in_=h1[:, kc * P:(kc + 1) * P],
                    identity=ident_bf[:BL, :BL],
                )
                nc.vector.tensor_copy(
                    out=h1T[:, kc * 16:kc * 16 + BL], in_=ps[:]
                )

            h1v = h1T[:].rearrange("p (k j s) -> p k j s", k=NK2DR, j=2)
            ps_h2 = psA.tile([BL, FC_H], f32, tag="psA")
            for kc2 in range(NK2DR):
                wt = stream_pool.tile([P, 2 * FC_H], fp8, tag="fcw")
                nc.sync.dma_start(out=wt[:], in_=d_fcw2[kc2])
                wv = wt[:].rearrange("p (j n) -> p j n", j=2)
                for n2 in range(2):
                    nc.tensor.matmul(
                        out=ps_h2[:, n2 * 512:(n2 + 1) * 512],
                        lhsT=h1v[:, kc2, :, :BL],
                        rhs=wv[:, :, n2 * 512:(n2 + 1) * 512],
                        start=(kc2 == 0),
                        stop=(kc2 == NK2DR - 1),
                        perf_mode=DR,
                    )
            h2 = hfin_pool.tile([BL, FC_H], bf16, tag="hfin")
            if has_fcb:
                fcb2_sb = hfin_pool.tile([BL, FC_H], f32, tag="fcb")
                nc.sync.dma_start(out=fcb2_sb[:], in_=d_fcb2[:])
                nc.vector.tensor_add(out=h2[:], in0=ps_h2[:], in1=fcb2_sb[:])
            else:
                nc.vector.tensor_copy(out=h2[:], in_=ps_h2[:])

            h2T = const_pool.tile([P, NKFC2 * 16], fp8, tag="h2T")
            for kc in range(NKFC2):
                ps = psA.tile([P, BL], bf16, tag="psA")
                nc.tensor.transpose(
                    out=ps[:],
                    in_=h2[:, kc * P:(kc + 1) * P],
                    identity=ident_bf[:BL, :BL],
                )
                nc.vector.tensor_copy(
                    out=h2T[:, kc * 16:kc * 16 + BL], in_=ps[:]
                )

            # FC3, software-pipelined: chunk ch's tanh/store tail is
            # emitted under chunk ch+1's matmuls.  The whole tail stays in
            # batch-major [BL, 1024] layout (OUT is [BL, FLAT]): no
            # transposes, and tanh reads the PSUM accumulator directly.
            nc.gpsimd.dma_start(out=vflat[:], in_=d_vflat[:])
            h2v = h2T[:].rearrange("p (k j s) -> p k j s", k=NK2DR, j=2)
            NCH = FLAT // FC_H  # 6
            ps_acc = [None] * NCH

            def fc3_tail(ch):
                cols = slice(ch * FC_H, (ch + 1) * FC_H)
                dch = hfin_pool.tile([BL, FC_H], f32, tag="dch",
                                     name=f"dch{ch}")
                if has_fcb:
                    fcb3_sb = hfin_pool.tile([BL, FC_H], f32, tag="fcb",
                                             name=f"fcb3_{ch}")
                    nc.gpsimd.dma_start(
                        out=fcb3_sb[:],
                        in_=d_fcb3[:, ch * FC_H:(ch + 1) * FC_H],
                    )
                    h3sb = hfin_pool.tile([BL, FC_H], f32, tag="hfin",
                                          name=f"h3sb{ch}")
                    nc.vector.tensor_add(
                        out=h3sb[:], in0=ps_acc[ch][:], in1=fcb3_sb[:]
                    )
                    nc.scalar.activation(out=dch[:], in_=h3sb[:],
                                         func=AF.Tanh)
                else:
                    nc.scalar.activation(out=dch[:], in_=ps_acc[ch][:],
                                         func=AF.Tanh)
                och = hfin_pool.tile([BL, FC_H], f32, tag="och",
                                     name=f"och{ch}")
                nc.vector.tensor_scalar_mul(
                    out=och[:], in0=dch[:], scalar1=0.1
                )
                nc.vector.tensor_add(
                    out=och[:], in0=och[:], in1=vflat[:, cols]
                )
                nc.sync.dma_start(out=d_out[:, cols], in_=och[:])

            for ch in range(NCH):
                ps = psA.tile([BL, FC_H], f32, tag="psA",
                              name=f"ps_fc3_{ch}")
                ps_acc[ch] = ps
                for kc2 in range(NK2DR):
                    wt = stream_pool.tile([P, 2 * FC_H], fp8, tag="fcw")
                    nc.sync.dma_start(out=wt[:], in_=d_fcw3[ch * NK2DR + kc2])
                    wv = wt[:].rearrange("p (j n) -> p j n", j=2)
                    for n2 in range(2):
                        nc.tensor.matmul(
                            out=ps[:, n2 * 512:(n2 + 1) * 512],
                            lhsT=h2v[:, kc2, :, :BL],
                            rhs=wv[:, :, n2 * 512:(n2 + 1) * 512],
                            start=(kc2 == 0),
                            stop=(kc2 == NK2DR - 1),
                            perf_mode=DR,
                        )
                if ch >= 1:
                    fc3_tail(ch - 1)
            fc3_tail(NCH - 1)

    nc.finalize()
    return nc


def build_in_maps(inputs):
    """Host prep + per-core input maps (exposed for testing)."""
    shared, per_core = _host_prep(inputs)
    key = (shared["HAS_BIAS1"], shared["HAS_BIAS2"], shared["HAS_BIAS3"],
           shared["HAS_FCB"])
    shared_arrays = {k: v for k, v in shared.items() if isinstance(v, np.ndarray)}
    in_maps = []
    for c in range(N_CORES):
        m = dict(shared_arrays)
        m.update(per_core[c])
        in_maps.append(m)
    return key, in_maps


def unpack_out(raw):
    return np.asarray(raw, np.float32).reshape(BL, V, 3)


def kernel(**inputs):
    key, in_maps = build_in_maps(inputs)
    if key not in _CACHE:
        _CACHE[key] = _build_program(*key)
    nc = _CACHE[key]

    from concourse.bass_utils import run_bass_kernel_spmd

    res = run_bass_kernel_spmd(nc, in_maps, list(range(N_CORES)))
    out = np.empty((B, V, 3), np.float32)
    for c in range(N_CORES):
        out[c * BL:(c + 1) * BL] = unpack_out(res.results[c]["OUT"])
    return out
